# revision 1
# baseline (speedup 1.0000x reference)
"""Trainium2 Bass kernel for scrambled-GQA sliding-window attention.

Head-parallel across 8 NeuronCores, two SPMD launches, no collectives:
  launch 1: QKV projection + RoPE + banded attention -> per-core y^T (2 heads)
  launch 2: output projection, sequence-parallel rows -> per-core 512 output rows

The torch-faithful "scrambled" reshapes in the reference are equivalent to
reinterpreting column slices of qkv = x @ W_attn:
  Q^T_h[d, m*256+t''] = qkv[t''*16+h, c_q(m)*128+d],  c_q(m)=m+2*(m//4), m in [0,16)
  K^T_h'[d, g*1024+u] = qkv[u*4+h', (6g+4)*128+d],    g in [0,4)
  V_h'[g*1024+u, d]   = qkv[u*4+h', (6g+5)*128+d]
Head h attends K/V block h' = h//4 over all rows with band |tq-tk| <= 1023.
RoPE applied to Q,K at position = row index (interleaved pairs).
"""

import math

import numpy as np

B, T, C = 1, 4096, 2048
NH, NKV, HD = 16, 4, 128
WINDOW = 1024
NCORES = 8
P = 128
KC = C // P            # 16 contraction chunks
NM = 16                # scramble chunks (m)
SCALE = 1.0 / math.sqrt(HD)

TQ = 512               # tq tile
NT = T // TQ           # 8 tiles per head
NBLK = T // P          # 32 tk blocks


def _cq(m):
    return m + 2 * (m // 4)


def _block_range(ti):
    b0 = 4 * ti
    return max(0, b0 - 8), min(NBLK - 1, b0 + 11)


def _mask_patterns():
    """Partial-band mask tiles keyed by D = tq0 - tkb (multiples of 128)."""
    ds = [640, 768, 896, 1024, -1024, -1152, -1280, -1408]
    tk = np.arange(P)[:, None]
    tq = np.arange(TQ)[None, :]
    masks = {}
    for d in ds:
        masks[d] = (np.abs(d + tq - tk) <= (WINDOW - 1)).astype(np.float32)
    return ds, masks


def host_prep(x, freqs_cis, W_attn, W_proj):
    """Build all per-core / shared numpy inputs for launch 1."""
    x = np.asarray(x, np.float32)
    freqs_cis = np.asarray(freqs_cis, np.float32)
    W_attn = np.asarray(W_attn, np.float32)

    xT = np.ascontiguousarray(x[0].T)            # (C, T) = (2048, 4096)

    # RoPE tables, (128, T): rows 2i,2i+1 = cos(ang[:, i]); sin signed.
    cos = np.repeat(freqs_cis[:, :, 0].T, 2, axis=0).astype(np.float32)   # (128, T)
    sin_base = freqs_cis[:, :, 1].T                                        # (64, T)
    sin = np.empty((P, T), np.float32)
    sin[0::2] = -sin_base
    sin[1::2] = sin_base

    # pair-swap matrix (symmetric): row 2i <-> row 2i+1
    mt = np.zeros((P, P), np.float32)
    ii = np.arange(0, P, 2)
    mt[ii, ii + 1] = 1.0
    mt[ii + 1, ii] = 1.0

    mask_ds, masks = _mask_patterns()
    masks_arr = np.ascontiguousarray(
        np.stack([masks[d] for d in mask_ds], axis=1))       # (128, 8, 512)

    wa3 = W_attn.reshape(KC, P, 24, HD)          # [kc][p][blk][d]
    wq = np.stack(
        [np.ascontiguousarray(
            wa3[:, :, _cq(m), :].transpose(1, 0, 2).reshape(P, KC * HD))
         for m in range(NM)]
    )                                            # (16, 128, 2048)
    wk = np.stack(
        [np.ascontiguousarray(
            wa3[:, :, 6 * g + 4, :].transpose(1, 0, 2).reshape(P, KC * HD))
         for g in range(NKV)]
    )                                            # (4, 128, 2048)
    wv = np.ascontiguousarray(
        np.concatenate([wa3[:, :, 6 * g + 5, :] for g in range(NKV)], axis=2)
        .transpose(1, 0, 2)
        .reshape(P, KC, NKV * HD)
    )                                            # (128, 16, 512)

    per_core = []
    for c in range(NCORES):
        hp = c // 2
        cols = np.concatenate([np.arange(256) * 16 + (2 * c + z) for z in (0, 1)])
        xq = np.ascontiguousarray(
            xT[:, cols].reshape(KC, P, 512).transpose(1, 0, 2))   # (128, 16, 512)
        ucols = np.arange(1024) * 4 + hp
        xkv = np.ascontiguousarray(
            xT[:, ucols].reshape(KC, P, 1024).transpose(1, 0, 2))  # (128, 16, 1024)
        per_core.append(
            dict(xq=xq, xkv=xkv, wq=wq, wk=wk, wv=wv, cos=cos, sin=sin,
                 mt=mt, ones=np.ones((P, P), np.float32), masks=masks_arr)
        )
    return per_core, mask_ds


def host_prep_proj(yT_full, W_proj):
    """yT_full: (16, 128, 4096) per-head transposed attention output."""
    W_proj = np.asarray(W_proj, np.float32)
    wp = np.ascontiguousarray(W_proj.reshape(NH, HD, C))     # (16, 128, 2048)
    per_core = []
    for c in range(NCORES):
        yt = np.ascontiguousarray(yT_full[:, :, c * 512:(c + 1) * 512])
        per_core.append(dict(yt=yt, wp=wp))
    return per_core


# ---------------------------------------------------------------------------
# numpy emulation of the exact device algorithm (validates all index math)
# ---------------------------------------------------------------------------

def emulate(x, freqs_cis, W_attn, W_proj):
    per_core, mask_ds = host_prep(x, freqs_cis, W_attn, W_proj)
    _, masks = _mask_patterns()
    yT_full = np.zeros((NH, P, T), np.float32)
    for c in range(NCORES):
        d = per_core[c]
        xq = d["xq"].transpose(1, 0, 2).reshape(C, 512)       # (2048, 512)
        xkv = d["xkv"].transpose(1, 0, 2).reshape(C, 1024)
        cos, sin, mt = d["cos"], d["sin"], d["mt"]
        qr = np.zeros((2, P, T), np.float32)
        for m in range(NM):
            wq_full = d["wq"][m].reshape(P, KC, HD).transpose(1, 0, 2).reshape(C, HD)
            qt = wq_full.T @ xq                               # (128, 512) [d,(z,t'')]
            qsw = mt @ qt
            c2 = np.concatenate([cos[:, m * 256:(m + 1) * 256]] * 2, axis=1)
            s2 = np.concatenate([sin[:, m * 256:(m + 1) * 256]] * 2, axis=1)
            qt = qt * c2 + qsw * s2
            qr[0, :, m * 256:(m + 1) * 256] = qt[:, :256]
            qr[1, :, m * 256:(m + 1) * 256] = qt[:, 256:]
        kr = np.zeros((P, T), np.float32)
        for g in range(NKV):
            wkg = d["wk"][g].reshape(P, KC, HD).transpose(1, 0, 2).reshape(C, HD)
            kt = wkg.T @ xkv                                  # (128, 1024)
            ksw = mt @ kt
            sl = slice(g * 1024, (g + 1) * 1024)
            kr[:, sl] = kt * cos[:, sl] + ksw * sin[:, sl]
        vall = np.zeros((P, 8, 512), np.float32)
        wv_full = d["wv"].transpose(1, 0, 2).reshape(C, 512)
        for ut in range(8):
            vall[:, ut, :] = xkv[:, ut * 128:(ut + 1) * 128].T @ wv_full
        for z in range(2):
            for ti in range(NT):
                blo, bhi = _block_range(ti)
                q_tile = qr[z, :, ti * TQ:(ti + 1) * TQ]
                y_acc = np.zeros((P, TQ), np.float32)
                s_acc = np.zeros((TQ,), np.float32)
                for b in range(blo, bhi + 1):
                    st = kr[:, b * P:(b + 1) * P].T @ q_tile  # (128tk, 512)
                    pt = np.exp(SCALE * st)
                    D = 512 * ti - 128 * b
                    if D in masks:
                        pt = pt * masks[D]
                    g, ub = b // 8, b % 8
                    vblk = vall[:, ub, g * HD:(g + 1) * HD]   # (128u, 128d)
                    y_acc += vblk.T @ pt
                    s_acc += pt.sum(axis=0)
                yT_full[2 * c + z, :, ti * TQ:(ti + 1) * TQ] = y_acc / s_acc[None, :]
    pc = host_prep_proj(yT_full, W_proj)
    outs = []
    for c in range(NCORES):
        yt, wp = pc[c]["yt"], pc[c]["wp"]
        acc = np.zeros((512, C), np.float32)
        for h in range(NH):
            acc += yt[h].T @ wp[h]
        outs.append(acc)
    return np.concatenate(outs, axis=0).reshape(B, T, C)


# ---------------------------------------------------------------------------
# Bass programs
# ---------------------------------------------------------------------------

def build_launch1():
    import concourse.bacc as bacc
    import concourse.mybir as mybir
    import concourse.tile as tile

    f32 = mybir.dt.float32
    f32r = mybir.dt.float32r
    MUL = mybir.AluOpType.mult
    ADD = mybir.AluOpType.add

    def fr(ap):
        return ap.bitcast(f32r)

    nc = bacc.Bacc("TRN2", target_bir_lowering=False, debug=False)

    xq_d = nc.dram_tensor("xq", (P, KC, 512), f32r, kind="ExternalInput")
    xkv_d = nc.dram_tensor("xkv", (P, KC, 1024), f32r, kind="ExternalInput")
    wq_d = nc.dram_tensor("wq", (NM, P, KC * HD), f32r, kind="ExternalInput")
    wk_d = nc.dram_tensor("wk", (NKV, P, KC * HD), f32r, kind="ExternalInput")
    wv_d = nc.dram_tensor("wv", (P, KC, 512), f32r, kind="ExternalInput")
    cos_d = nc.dram_tensor("cos", (P, T), f32, kind="ExternalInput")
    sin_d = nc.dram_tensor("sin", (P, T), f32, kind="ExternalInput")
    mt_d = nc.dram_tensor("mt", (P, P), f32r, kind="ExternalInput")
    ones_d = nc.dram_tensor("ones", (P, P), f32r, kind="ExternalInput")
    masks_d = nc.dram_tensor("masks", (P, 8, TQ), f32, kind="ExternalInput")
    yt_d = nc.dram_tensor("yt", (2, P, T), f32, kind="ExternalOutput")

    mask_ds, _ = _mask_patterns()
    mask_idx = {d: i for i, d in enumerate(mask_ds)}

    with tile.TileContext(nc) as tc:
        with tc.tile_pool(name="persist", bufs=1) as persist:
            qr = persist.tile([P, 2, T], f32r, tag="qr", name="qr")
            kr = persist.tile([P, T], f32r, tag="kr", name="kr")
            vall = persist.tile([P, 8, 512], f32r, tag="vall", name="vall")
            ones = persist.tile([P, P], f32r, tag="ones", name="ones")
            mt_s = persist.tile([P, P], f32r, tag="mt", name="mt_s")
            nc.sync.dma_start(ones[:], ones_d.ap())
            nc.sync.dma_start(mt_s[:], mt_d.ap())

            def rope(wpool, ppool, sb, tsl, n, out_ap, view, vtab):
                """out = sb*cos + (M @ sb)*sin; sb is flat (128, 512) SBUF.
                cos/sin slices [tsl] of width n are streamed from DRAM."""
                ctab = wpool.tile([P, TQ], f32, tag="ctab", bufs=2, name="ctab")
                nc.sync.dma_start(ctab[:, :n], cos_d.ap()[:, tsl])
                stab = wpool.tile([P, TQ], f32, tag="stab", bufs=2, name="stab")
                nc.sync.dma_start(stab[:, :n], sin_d.ap()[:, tsl])
                sw = ppool.tile([P, TQ], f32, tag="rope_sw", bufs=2, name="rope_sw")
                nc.tensor.matmul(sw, mt_s[:], sb, start=True, stop=True)
                t1 = wpool.tile([P, TQ], f32, tag="rope_t1", bufs=1, name="rope_t1")
                nc.vector.tensor_tensor(view(t1), view(sb), vtab(ctab), MUL)
                t2 = wpool.tile([P, TQ], f32, tag="rope_t2", bufs=1, name="rope_t2")
                nc.vector.tensor_tensor(view(t2), view(sw), vtab(stab), MUL)
                nc.vector.tensor_tensor(out_ap, view(t1), view(t2), ADD)

            with tc.tile_pool(name="wstream", bufs=2) as wsp, \
                 tc.tile_pool(name="qp", bufs=1) as qp:
                xq_s = qp.tile([P, KC, 512], f32r, tag="xq", bufs=1,
                               name="xq_s")

                with tc.tile_pool(name="xkvp", bufs=1) as xkvp:
                    xkv_s = xkvp.tile([P, KC, 1024], f32r, tag="xkv", name="xkv_s")

                    # ---- V (kc-outer: PE starts after first ~0.75MB of DMA) ----
                    with tc.tile_pool(name="vp", bufs=4) as vp, \
                         tc.tile_pool(name="vps", bufs=8, space="PSUM") as vps:
                        vpss = [vps.tile([P, TQ], f32, tag="vpsum",
                                         name="vpsum") for _ in range(8)]
                        for kc in range(KC):
                            nc.sync.dma_start(xkv_s[:, kc], xkv_d.ap()[:, kc])
                            wv_c = vp.tile([P, 1, 512], f32r, tag="wv",
                                           bufs=3, name="wv_c")
                            nc.sync.dma_start(wv_c[:, 0], wv_d.ap()[:, kc])
                            for ut in range(8):
                                nc.tensor.matmul(
                                    vpss[ut],
                                    xkv_s[:, kc, ut * P:(ut + 1) * P],
                                    wv_c[:, 0],
                                    start=(kc == 0), stop=(kc == KC - 1))
                        for ut in range(8):
                            nc.scalar.copy(vall[:, ut], vpss[ut])
                    kqps = tc.alloc_tile_pool(name="kqps", bufs=2,
                                              space="PSUM")

                    # prefetch wk while V finishes; cos/sin after
                    wk_ss = []
                    for g in range(NKV):
                        wk_s = wsp.tile([P, KC, HD], f32r, tag="w", bufs=3,
                                        name="wk_s")
                        nc.sync.dma_start(
                            wk_s[:],
                            wk_d.ap()[g].rearrange("p (kc d) -> p kc d", d=HD))
                        wk_ss.append(wk_s)
                    for kc in range(KC):
                        nc.sync.dma_start(xq_s[:, kc], xq_d.ap()[:, kc])

                    # ---- K ----
                    for g in range(NKV):
                        for ut in range(2):
                            ps = kqps.tile([P, TQ], f32, tag="qkv",
                                           name="kpsum")
                            for kc in range(KC):
                                nc.tensor.matmul(
                                    ps, wk_ss[g][:, kc],
                                    xkv_s[:, kc, ut * TQ:(ut + 1) * TQ],
                                    start=(kc == 0), stop=(kc == KC - 1))
                            ksb = wsp.tile([P, TQ], f32r, tag="sbr", bufs=2,
                                           name="ksb")
                            nc.scalar.copy(ksb, ps)
                            sl = slice(g * 1024 + ut * TQ,
                                       g * 1024 + (ut + 1) * TQ)
                            rope(wsp, kqps, ksb, sl, TQ, kr[:, sl],
                                 lambda a: a, lambda tb: tb[:, :TQ])

                # ---- Q ----
                if True:
                    for m in range(NM):
                        wq_s = wsp.tile([P, KC, HD], f32r, tag="w", bufs=3,
                                        name="wq_s")
                        nc.sync.dma_start(
                            wq_s[:],
                            wq_d.ap()[m].rearrange("p (kc d) -> p kc d", d=HD))
                        ps = kqps.tile([P, TQ], f32, tag="qkv", name="qpsum")
                        for kc in range(KC):
                            nc.tensor.matmul(
                                ps, wq_s[:, kc], xq_s[:, kc],
                                start=(kc == 0), stop=(kc == KC - 1))
                        qsb = wsp.tile([P, TQ], f32r, tag="sbr", bufs=2, name="qsb")
                        nc.scalar.copy(qsb, ps)
                        rope(wsp, kqps, qsb,
                             slice(m * 256, (m + 1) * 256), 256,
                             qr[:, :, m * 256:(m + 1) * 256],
                             lambda a: a.rearrange("p (z t) -> p z t", z=2),
                             lambda tb: tb[:, None, :256].broadcast_to(
                                 (P, 2, 256)))

                kqps.release()

            # ---- attention ----
            with tc.tile_pool(name="asb", bufs=2) as asb, \
                 tc.tile_pool(name="ptp", bufs=4) as ptp, \
                 tc.tile_pool(name="sps", bufs=2, space="PSUM") as sps, \
                 tc.tile_pool(name="yps", bufs=2, space="PSUM") as yps, \
                 tc.tile_pool(name="sups", bufs=2, space="PSUM") as sups:
                masks_s = asb.tile([P, 8, TQ], f32, tag="masks", bufs=1,
                                   name="masks_s")
                nc.sync.dma_start(masks_s[:], masks_d.ap())
                for z in range(2):
                    for ti in range(NT):
                        blo, bhi = _block_range(ti)
                        bs = list(range(blo, bhi + 1))
                        q_rhs = qr[:, z, ti * TQ:(ti + 1) * TQ]
                        y_ps = yps.tile([P, TQ], f32, tag="y", name="y_ps")
                        s_ps = sups.tile([1, TQ], f32, tag="sums", name="s_ps")
                        for gi in range(0, len(bs), 2):
                            grp = bs[gi:gi + 2]
                            ng = len(grp)
                            st_ps = sps.tile([P, 2 * TQ], f32, tag="st",
                                             name="st_ps")
                            for j, b in enumerate(grp):
                                nc.tensor.matmul(
                                    st_ps[:, j * TQ:(j + 1) * TQ],
                                    kr[:, b * P:(b + 1) * P], q_rhs,
                                    start=True, stop=True)
                            pt = ptp.tile([P, 2 * TQ], f32r, tag="pt", name="pt")
                            nc.scalar.activation(
                                pt[:, :ng * TQ], st_ps[:, :ng * TQ],
                                mybir.ActivationFunctionType.Exp, scale=SCALE)
                            for j, b in enumerate(grp):
                                D = 512 * ti - 128 * b
                                if D in mask_idx:
                                    nc.vector.tensor_tensor(
                                        pt[:, j * TQ:(j + 1) * TQ],
                                        pt[:, j * TQ:(j + 1) * TQ],
                                        masks_s[:, mask_idx[D]], MUL)
                            for j, b in enumerate(grp):
                                first = (gi + j == 0)
                                last = (gi + j == len(bs) - 1)
                                g, ub = b // 8, b % 8
                                nc.tensor.matmul(
                                    y_ps, vall[:, ub, g * HD:(g + 1) * HD],
                                    pt[:, j * TQ:(j + 1) * TQ],
                                    start=first, stop=last)
                                nc.tensor.matmul(
                                    s_ps, ones[:, 0:1],
                                    pt[:, j * TQ:(j + 1) * TQ],
                                    start=first, stop=last)
                        r_sb = asb.tile([1, TQ], f32, tag="rsb", name="r_sb")
                        nc.vector.reciprocal_approx_fast(r_sb, s_ps)
                        rb_sb = asb.tile([P, TQ], f32, tag="rbsb", name="rb_sb")
                        nc.gpsimd.partition_broadcast(rb_sb, r_sb)
                        y_sb = asb.tile([P, TQ], f32, tag="ysb", name="y_sb")
                        nc.vector.tensor_tensor(y_sb, y_ps, rb_sb, MUL)
                        nc.sync.dma_start(
                            yt_d.ap()[z, :, ti * TQ:(ti + 1) * TQ], y_sb)

    nc.compile()
    return nc


def build_launch2():
    import concourse.bacc as bacc
    import concourse.mybir as mybir
    import concourse.tile as tile

    f32 = mybir.dt.float32
    f32r = mybir.dt.float32r

    def fr(ap):
        return ap.bitcast(f32r)

    nc = bacc.Bacc("TRN2", target_bir_lowering=False, debug=False)
    yt_d = nc.dram_tensor("yt", (NH, P, 512), f32r, kind="ExternalInput")
    wp_d = nc.dram_tensor("wp", (NH, P, C), f32r, kind="ExternalInput")
    out_d = nc.dram_tensor("out", (512, C), f32, kind="ExternalOutput")

    with tile.TileContext(nc) as tc:
        with tc.tile_pool(name="sb", bufs=2) as sb, \
             tc.tile_pool(name="ps", bufs=8, space="PSUM") as psp:
            yt_s = sb.tile([P, NH, 512], f32r, tag="yt", bufs=1, name="yt_s")
            wp_s = sb.tile([P, NH, C], f32r, tag="wp", bufs=1, name="wp_s")
            for h in range(NH):
                nc.sync.dma_start(yt_s[:, h], yt_d.ap()[h])
                nc.sync.dma_start(wp_s[:, h], wp_d.ap()[h])
            # h-outer accumulation: 8 resident psum tiles per ct-half so the
            # first matmuls only wait on wp[0]/yt[0]; lhsT shared across ct.
            for ch in range(2):
                pss = [[psp.tile([P, 512], f32, tag="ps", name="ps")
                        for _ in range(2)] for _ in range(4)]
                for h in range(NH):
                    for tt in range(4):
                        for c2 in range(2):
                            ct = ch * 2 + c2
                            nc.tensor.matmul(
                                pss[tt][c2], yt_s[:, h, tt * P:(tt + 1) * P],
                                wp_s[:, h, ct * 512:(ct + 1) * 512],
                                start=(h == 0), stop=(h == NH - 1))
                for tt in range(4):
                    for c2 in range(2):
                        ct = ch * 2 + c2
                        o_sb = sb.tile([P, 512], f32, tag="osb", bufs=4,
                                       name="o_sb")
                        nc.vector.tensor_copy(o_sb, pss[tt][c2])
                        nc.sync.dma_start(
                            out_d.ap()[tt * P:(tt + 1) * P,
                                       ct * 512:(ct + 1) * 512],
                            o_sb)
    nc.compile()
    return nc


_cache = {}


def kernel(x, freqs_cis, W_attn, W_proj, _trace=False, _timing=None):
    from concourse.bass_utils import run_bass_kernel_spmd

    per_core, _ = host_prep(x, freqs_cis, W_attn, W_proj)

    if "l1" not in _cache:
        _cache["l1"] = build_launch1()
    if "l2" not in _cache:
        _cache["l2"] = build_launch2()

    kw = dict(trace=True, trace_cores=list(range(NCORES))) if _trace else {}
    res1 = run_bass_kernel_spmd(_cache["l1"], per_core, list(range(NCORES)), **kw)
    yT_full = np.empty((NH, P, T), np.float32)
    for c in range(NCORES):
        yT_full[2 * c:2 * c + 2] = res1.results[c]["yt"]

    pc2 = host_prep_proj(yT_full, W_proj)
    res2 = run_bass_kernel_spmd(_cache["l2"], pc2, list(range(NCORES)), **kw)
    out = np.concatenate([res2.results[c]["out"] for c in range(NCORES)], axis=0)

    if _timing is not None:
        _timing["l1_ns"] = res1.exec_time_ns
        _timing["l2_ns"] = res2.exec_time_ns
        _timing["res1"] = res1
        _timing["res2"] = res2
    return out.reshape(B, T, C)



# revision 7
# speedup vs baseline: 1.2371x; 1.2371x over previous
"""Trainium2 Bass kernel for scrambled-GQA sliding-window attention.

Head-parallel across 8 NeuronCores, two SPMD launches, no collectives:
  launch 1: QKV projection + RoPE + banded attention -> per-core y^T (2 heads)
  launch 2: output projection, sequence-parallel rows -> per-core 512 output rows

All matmul operands are fp16 (1 col/cycle on PE, FWL weight loads, half DMA).
The two heads per core share K/V blocks, so score matmuls pair them as one
1024-column fp16 matmul (one PSUM bank).  Softmax denominators come from a
DVE-accumulated sum of exp tiles plus a single ones-matmul per (ti) tile
instead of one per block.  Q projection is interleaved with attention tiles
(tile ti only needs m-chunks 2ti, 2ti+1) to keep PE dense while ACT runs exp.

The torch-faithful "scrambled" reshapes in the reference are equivalent to
reinterpreting column slices of qkv = x @ W_attn:
  Q^T_h[d, m*256+t''] = qkv[t''*16+h, c_q(m)*128+d],  c_q(m)=m+2*(m//4), m in [0,16)
  K^T_h'[d, g*1024+u] = qkv[u*4+h', (6g+4)*128+d],    g in [0,4)
  V_h'[g*1024+u, d]   = qkv[u*4+h', (6g+5)*128+d]
Head h attends K/V block h' = h//4 over all rows with band |tq-tk| <= 1023.
RoPE applied to Q,K at position = row index (interleaved pairs).
"""

import math

import numpy as np

B, T, C = 1, 4096, 2048
NH, NKV, HD = 16, 4, 128
WINDOW = 1024
NCORES = 8
P = 128
KC = C // P            # 16 contraction chunks
NM = 16                # scramble chunks (m)
SCALE = 1.0 / math.sqrt(HD)

TQ = 512               # tq tile
NT = T // TQ           # 8 tiles per head
NBLK = T // P          # 32 tk blocks


def _cq(m):
    return m + 2 * (m // 4)


def _block_range(ti):
    b0 = 4 * ti
    return max(0, b0 - 8), min(NBLK - 1, b0 + 11)


def _mask_patterns():
    """Partial-band mask tiles keyed by D = tq0 - tkb (multiples of 128)."""
    ds = [640, 768, 896, 1024, -1024, -1152, -1280, -1408]
    tk = np.arange(P)[:, None]
    tq = np.arange(TQ)[None, :]
    masks = {}
    for d in ds:
        masks[d] = (np.abs(d + tq - tk) <= (WINDOW - 1)).astype(np.float32)
    return ds, masks


def host_prep(x, freqs_cis, W_attn, W_proj):
    """Build all per-core / shared numpy inputs for launch 1 (fp16)."""
    x = np.asarray(x, np.float32)
    freqs_cis = np.asarray(freqs_cis, np.float32)
    W_attn = np.asarray(W_attn, np.float32)

    xT = np.ascontiguousarray(x[0].T).astype(np.float16)    # (C, T)

    # RoPE tables, (128, T): rows 2i,2i+1 = cos(ang[:, i]); sin signed.
    cos = np.repeat(freqs_cis[:, :, 0].T, 2, axis=0).astype(np.float16)
    sin_base = freqs_cis[:, :, 1].T                          # (64, T)
    sin = np.empty((P, T), np.float32)
    sin[0::2] = -sin_base
    sin[1::2] = sin_base
    sin = sin.astype(np.float16)

    # pair-swap matrix (symmetric): row 2i <-> row 2i+1
    mt = np.zeros((P, P), np.float16)
    ii = np.arange(0, P, 2)
    mt[ii, ii + 1] = 1.0
    mt[ii + 1, ii] = 1.0

    mask_ds, masks = _mask_patterns()
    masks_arr = np.ascontiguousarray(
        np.stack([masks[d] for d in mask_ds], axis=1)).astype(np.float16)

    wa3 = W_attn.reshape(KC, P, 24, HD)          # [kc][p][blk][d]
    wq = np.stack(
        [np.ascontiguousarray(
            wa3[:, :, _cq(m), :].transpose(1, 0, 2).reshape(P, KC * HD))
         for m in range(NM)]
    ).astype(np.float16)                         # (16, 128, 2048)
    wk = np.stack(
        [np.ascontiguousarray(
            wa3[:, :, 6 * g + 4, :].transpose(1, 0, 2).reshape(P, KC * HD))
         for g in range(NKV)]
    ).astype(np.float16)                         # (4, 128, 2048)
    wv = np.ascontiguousarray(
        np.concatenate([wa3[:, :, 6 * g + 5, :] for g in range(NKV)], axis=2)
        .transpose(1, 0, 2)
        .reshape(P, KC, NKV * HD)
    ).astype(np.float16)                         # (128, 16, 512)

    per_core = []
    for c in range(NCORES):
        hp = c // 2
        cols = np.concatenate([np.arange(256) * 16 + (2 * c + z) for z in (0, 1)])
        xq = np.ascontiguousarray(
            xT[:, cols].reshape(KC, P, 512).transpose(1, 0, 2))   # (128, 16, 512)
        ucols = np.arange(1024) * 4 + hp
        xkv = np.ascontiguousarray(
            xT[:, ucols].reshape(KC, P, 1024).transpose(1, 0, 2))  # (128, 16, 1024)
        per_core.append(
            dict(xq=xq, xkv=xkv, wq=wq, wk=wk, wv=wv, cos=cos, sin=sin,
                 mt=mt, masks=masks_arr)
        )
    return per_core, mask_ds


def host_prep_proj(yT_full, W_proj):
    """yT_full: (16, 128, 4096) fp16 per-head transposed attention output."""
    W_proj = np.asarray(W_proj, np.float32)
    wp = np.ascontiguousarray(W_proj.reshape(NH, HD, C)).astype(np.float16)
    per_core = []
    for c in range(NCORES):
        yt = np.ascontiguousarray(yT_full[:, :, c * 512:(c + 1) * 512])
        per_core.append(dict(yt=yt, wp=wp))
    return per_core


# ---------------------------------------------------------------------------
# numpy emulation of the exact device algorithm (validates all index math)
# ---------------------------------------------------------------------------

def emulate(x, freqs_cis, W_attn, W_proj):
    per_core, mask_ds = host_prep(x, freqs_cis, W_attn, W_proj)
    _, masks = _mask_patterns()
    yT_full = np.zeros((NH, P, T), np.float32)
    for c in range(NCORES):
        d = per_core[c]
        xq = d["xq"].astype(np.float32).transpose(1, 0, 2).reshape(C, 512)
        xkv = d["xkv"].astype(np.float32).transpose(1, 0, 2).reshape(C, 1024)
        cos = d["cos"].astype(np.float32)
        sin = d["sin"].astype(np.float32)
        mt = d["mt"].astype(np.float32)
        qr = np.zeros((2, P, T), np.float32)
        for m in range(NM):
            wq_full = d["wq"][m].astype(np.float32).reshape(P, KC, HD)\
                .transpose(1, 0, 2).reshape(C, HD)
            qt = wq_full.T @ xq                               # (128, 512)
            qsw = mt @ qt
            c2 = np.concatenate([cos[:, m * 256:(m + 1) * 256]] * 2, axis=1)
            s2 = np.concatenate([sin[:, m * 256:(m + 1) * 256]] * 2, axis=1)
            qt = qt * c2 + qsw * s2
            qr[0, :, m * 256:(m + 1) * 256] = qt[:, :256]
            qr[1, :, m * 256:(m + 1) * 256] = qt[:, 256:]
        kr = np.zeros((P, T), np.float32)
        for g in range(NKV):
            wkg = d["wk"][g].astype(np.float32).reshape(P, KC, HD)\
                .transpose(1, 0, 2).reshape(C, HD)
            kt = wkg.T @ xkv                                  # (128, 1024)
            ksw = mt @ kt
            sl = slice(g * 1024, (g + 1) * 1024)
            kr[:, sl] = kt * cos[:, sl] + ksw * sin[:, sl]
        vall = np.zeros((P, 8, 512), np.float32)
        wv_full = d["wv"].astype(np.float32).transpose(1, 0, 2).reshape(C, 512)
        for ut in range(8):
            vall[:, ut, :] = xkv[:, ut * 128:(ut + 1) * 128].T @ wv_full
        for z in range(2):
            for ti in range(NT):
                blo, bhi = _block_range(ti)
                q_tile = qr[z, :, ti * TQ:(ti + 1) * TQ]
                y_acc = np.zeros((P, TQ), np.float32)
                s_acc = np.zeros((TQ,), np.float32)
                for b in range(blo, bhi + 1):
                    st = kr[:, b * P:(b + 1) * P].T @ q_tile  # (128tk, 512)
                    pt = np.exp(SCALE * st)
                    D = 512 * ti - 128 * b
                    if D in masks:
                        pt = pt * masks[D]
                    g, ub = b // 8, b % 8
                    vblk = vall[:, ub, g * HD:(g + 1) * HD]   # (128u, 128d)
                    y_acc += vblk.T @ pt
                    s_acc += pt.sum(axis=0)
                yT_full[2 * c + z, :, ti * TQ:(ti + 1) * TQ] = y_acc / s_acc[None, :]
    pc = host_prep_proj(yT_full.astype(np.float16), W_proj)
    outs = []
    for c in range(NCORES):
        yt = pc[c]["yt"].astype(np.float32)
        wp = pc[c]["wp"].astype(np.float32)
        acc = np.zeros((512, C), np.float32)
        for h in range(NH):
            acc += yt[h].T @ wp[h]
        outs.append(acc)
    return np.concatenate(outs, axis=0).reshape(B, T, C)


# ---------------------------------------------------------------------------
# Bass programs
# ---------------------------------------------------------------------------

def build_launch1():
    import concourse.bacc as bacc
    import concourse.mybir as mybir
    import concourse.tile as tile

    import concourse.bass_isa as bass_isa

    f16 = mybir.dt.float16
    f32 = mybir.dt.float32
    MUL = mybir.AluOpType.mult
    ADD = mybir.AluOpType.add
    EXP = mybir.ActivationFunctionType.Exp

    nc = bacc.Bacc("TRN2", target_bir_lowering=False, debug=False)

    xq_d = nc.dram_tensor("xq", (P, KC, 512), f16, kind="ExternalInput")
    xkv_d = nc.dram_tensor("xkv", (P, KC, 1024), f16, kind="ExternalInput")
    wq_d = nc.dram_tensor("wq", (NM, P, KC * HD), f16, kind="ExternalInput")
    wk_d = nc.dram_tensor("wk", (NKV, P, KC * HD), f16, kind="ExternalInput")
    wv_d = nc.dram_tensor("wv", (P, KC, 512), f16, kind="ExternalInput")
    cos_d = nc.dram_tensor("cos", (P, T), f16, kind="ExternalInput")
    sin_d = nc.dram_tensor("sin", (P, T), f16, kind="ExternalInput")
    mt_d = nc.dram_tensor("mt", (P, P), f16, kind="ExternalInput")
    masks_d = nc.dram_tensor("masks", (P, 8, TQ), f16, kind="ExternalInput")
    yt_d = nc.dram_tensor("yt", (2, P, T), f16, kind="ExternalOutput")

    mask_ds, _ = _mask_patterns()
    mask_idx = {d: i for i, d in enumerate(mask_ds)}

    with tile.TileContext(nc) as tc:
        with tc.tile_pool(name="persist", bufs=1) as persist:
            qr = persist.tile([P, NT, 2, TQ], f16, tag="qr", name="qr")
            kr = persist.tile([P, T], f16, tag="kr", name="kr")
            vall = persist.tile([P, 8, TQ], f16, tag="vall", name="vall")
            mt_s = persist.tile([P, P], f16, tag="mt", name="mt_s")
            masks_s = persist.tile([P, 8, TQ], f16, tag="masks", name="masks_s")
            xq_s = persist.tile([P, KC, 512], f16, tag="xq", name="xq_s")
            xkv_s = persist.tile([P, KC, 1024], f16, tag="xkv", name="xkv_s")
            nc.sync.dma_start(mt_s[:], mt_d.ap())
            nc.sync.dma_start(masks_s[:], masks_d.ap())

            with tc.tile_pool(name="wstream", bufs=2) as wsp:

                # ---- V (kc-outer: PE starts after first small DMA) ----
                with tc.tile_pool(name="vps", bufs=8, space="PSUM") as vps:
                    vpss = [vps.tile([P, TQ], f32, tag="vpsum",
                                     name="vpsum") for _ in range(8)]
                    for kc in range(KC):
                        nc.sync.dma_start(xkv_s[:, kc], xkv_d.ap()[:, kc])
                        wv_c = wsp.tile([P, 1, 512], f16, tag="wv",
                                        bufs=3, name="wv_c")
                        nc.sync.dma_start(wv_c[:, 0], wv_d.ap()[:, kc])
                        for ut in range(8):
                            nc.tensor.matmul(
                                vpss[ut],
                                xkv_s[:, kc, ut * P:(ut + 1) * P],
                                wv_c[:, 0],
                                start=(kc == 0), stop=(kc == KC - 1))
                    for ut in range(8):
                        nc.scalar.copy(vall[:, ut], vpss[ut])

                # prefetch wk while V finishes; xq after
                wk_ss = []
                for g in range(NKV):
                    wk_s = wsp.tile([P, KC, HD], f16, tag="wk", bufs=4,
                                    name="wk_s")
                    nc.sync.dma_start(
                        wk_s[:],
                        wk_d.ap()[g].rearrange("p (kc d) -> p kc d", d=HD))
                    wk_ss.append(wk_s)
                for kc in range(KC):
                    nc.sync.dma_start(xq_s[:, kc], xq_d.ap()[:, kc])

                def rope(ppool, sw_tag, sw_bufs, sb, tsl, n, out_ap, view, vtab):
                    """out = sb*cos + (M @ sb)*sin; sb is flat (128, 512) SBUF."""
                    ctab = wsp.tile([P, TQ], f16, tag="ctab", bufs=2, name="ctab")
                    nc.sync.dma_start(ctab[:, :n], cos_d.ap()[:, tsl])
                    stab = wsp.tile([P, TQ], f16, tag="stab", bufs=2, name="stab")
                    nc.sync.dma_start(stab[:, :n], sin_d.ap()[:, tsl])
                    sw = ppool.tile([P, TQ], f32, tag=sw_tag, bufs=sw_bufs,
                                    name=sw_tag)
                    nc.tensor.matmul(sw, mt_s[:], sb, start=True, stop=True)
                    t1 = wsp.tile([P, TQ], f16, tag="rope_t1", bufs=2,
                                  name="rope_t1")
                    nc.vector.tensor_tensor(view(t1), view(sb), vtab(ctab), MUL)
                    t2 = wsp.tile([P, TQ], f16, tag="rope_t2", bufs=2,
                                  name="rope_t2")
                    nc.vector.tensor_tensor(view(t2), view(sw), vtab(stab), MUL)
                    nc.vector.tensor_tensor(out_ap, view(t1), view(t2), ADD)

                # ---- K ----
                with tc.tile_pool(name="kqp", bufs=1, space="PSUM") as kqps:
                    for g in range(NKV):
                        for ut in range(2):
                            ps = kqps.tile([P, TQ], f32, tag="kps", bufs=2,
                                           name="kpsum")
                            for kc in range(KC):
                                nc.tensor.matmul(
                                    ps, wk_ss[g][:, kc],
                                    xkv_s[:, kc, ut * TQ:(ut + 1) * TQ],
                                    start=(kc == 0), stop=(kc == KC - 1))
                            ksb = wsp.tile([P, TQ], f16, tag="sbr", bufs=2,
                                           name="ksb")
                            nc.scalar.copy(ksb, ps)
                            sl = slice(g * 1024 + ut * TQ,
                                       g * 1024 + (ut + 1) * TQ)
                            rope(kqps, "ksw", 2, ksb, sl, TQ, kr[:, sl],
                                 lambda a: a, lambda tb: tb[:, :TQ])

                # ---- interleaved Q projection + attention ----
                with tc.tile_pool(name="aps", bufs=1, space="PSUM") as aps, \
                     tc.tile_pool(name="asb", bufs=2) as asb:
                    for ti in range(NT):
                        for mh in range(2):
                            m = 2 * ti + mh
                            wq_s = wsp.tile([P, KC, HD], f16, tag="wq", bufs=3,
                                            name="wq_s")
                            nc.sync.dma_start(
                                wq_s[:],
                                wq_d.ap()[m].rearrange("p (kc d) -> p kc d",
                                                       d=HD))
                            qps = aps.tile([P, TQ], f32, tag="qps", bufs=1,
                                           name="qpsum")
                            for kc in range(KC):
                                nc.tensor.matmul(
                                    qps, wq_s[:, kc], xq_s[:, kc],
                                    start=(kc == 0), stop=(kc == KC - 1))
                            qsb = wsp.tile([P, TQ], f16, tag="sbr", bufs=2,
                                           name="qsb")
                            nc.vector.tensor_copy(qsb, qps)
                            rope(aps, "qsw", 1, qsb,
                                 slice(m * 256, (m + 1) * 256), 256,
                                 qr[:, ti, :, mh * 256:mh * 256 + 256],
                                 lambda a: a.rearrange("p (z t) -> p z t", z=2),
                                 lambda tb: tb[:, None, :256].broadcast_to(
                                     (P, 2, 256)))

                        blo, bhi = _block_range(ti)
                        bs = (list(range(4 * ti, bhi + 1))
                              + list(range(blo, 4 * ti)))
                        nb = len(bs)
                        q_rhs = qr[:, ti]                     # (128, 2, 512)
                        y_ps = aps.tile([P, 2, TQ], f32, tag="y", bufs=1,
                                        name="y_ps")
                        acc = asb.tile([P, 2, TQ], f16, tag="acc", bufs=2,
                                       name="acc")
                        for j, b in enumerate(bs):
                            st = aps.tile([P, 2, TQ], f32, tag="st", bufs=2,
                                          name="st_ps")
                            for z in range(2):
                                nc.tensor.matmul(
                                    st[:, z], kr[:, b * P:(b + 1) * P],
                                    q_rhs[:, z], start=True, stop=True)
                            pt = asb.tile([P, 2, TQ], f16, tag="pt", bufs=3,
                                          name="pt")
                            nc.scalar.activation(pt, st, EXP, scale=SCALE)
                            D = 512 * ti - 128 * b
                            if D in mask_idx:
                                nc.vector.tensor_tensor(
                                    pt, pt,
                                    masks_s[:, mask_idx[D]][:, None, :]
                                    .broadcast_to((P, 2, TQ)),
                                    MUL)
                            if j == 0:
                                nc.vector.tensor_copy(acc, pt)
                            else:
                                nc.vector.tensor_tensor(acc, acc, pt, ADD)
                            g, ub = b // 8, b % 8
                            for z in range(2):
                                nc.tensor.matmul(
                                    y_ps[:, z],
                                    vall[:, ub, g * HD:(g + 1) * HD],
                                    pt[:, z],
                                    start=(j == 0), stop=(j == nb - 1))
                        s_bc = asb.tile([P, 2, TQ], f32, tag="sbc", bufs=2,
                                        name="s_bc")
                        nc.gpsimd.partition_all_reduce(
                            s_bc, acc, channels=P,
                            reduce_op=bass_isa.ReduceOp.add)
                        rb_sb = asb.tile([P, 2, TQ], f32, tag="rb", bufs=2,
                                         name="rb_sb")
                        nc.vector.reciprocal_approx_fast(rb_sb, s_bc)
                        y_sb = asb.tile([P, 2, TQ], f16, tag="ysb", bufs=2,
                                        name="y_sb")
                        nc.vector.tensor_tensor(y_sb, y_ps, rb_sb, MUL)
                        for z in range(2):
                            nc.sync.dma_start(
                                yt_d.ap()[z, :, ti * TQ:(ti + 1) * TQ],
                                y_sb[:, z])

    nc.compile()
    return nc


def build_launch2():
    import concourse.bacc as bacc
    import concourse.mybir as mybir
    import concourse.tile as tile

    f16 = mybir.dt.float16
    f32 = mybir.dt.float32

    nc = bacc.Bacc("TRN2", target_bir_lowering=False, debug=False)
    yt_d = nc.dram_tensor("yt", (NH, P, 512), f16, kind="ExternalInput")
    wp_d = nc.dram_tensor("wp", (NH, P, C), f16, kind="ExternalInput")
    out_d = nc.dram_tensor("out", (512, C), f32, kind="ExternalOutput")

    with tile.TileContext(nc) as tc:
        with tc.tile_pool(name="sb", bufs=2) as sb, \
             tc.tile_pool(name="ps", bufs=8, space="PSUM") as psp:
            yt_s = sb.tile([P, NH, 512], f16, tag="yt", bufs=1, name="yt_s")
            wp_s = sb.tile([P, NH, C], f16, tag="wp", bufs=1, name="wp_s")
            for h in range(NH):
                nc.sync.dma_start(yt_s[:, h], yt_d.ap()[h])
                nc.sync.dma_start(wp_s[:, h], wp_d.ap()[h])
            # h-outer accumulation: 8 resident psum tiles per ct-half so the
            # first matmuls only wait on wp[0]/yt[0]; lhsT shared across ct.
            for ch in range(2):
                pss = [[psp.tile([P, 512], f32, tag="ps", name="ps")
                        for _ in range(2)] for _ in range(4)]
                for h in range(NH):
                    for tt in range(4):
                        for c2 in range(2):
                            ct = ch * 2 + c2
                            nc.tensor.matmul(
                                pss[tt][c2], yt_s[:, h, tt * P:(tt + 1) * P],
                                wp_s[:, h, ct * 512:(ct + 1) * 512],
                                start=(h == 0), stop=(h == NH - 1))
                for tt in range(4):
                    for c2 in range(2):
                        ct = ch * 2 + c2
                        o_sb = sb.tile([P, 512], f32, tag="osb", bufs=4,
                                       name="o_sb")
                        nc.vector.tensor_copy(o_sb, pss[tt][c2])
                        nc.sync.dma_start(
                            out_d.ap()[tt * P:(tt + 1) * P,
                                       ct * 512:(ct + 1) * 512],
                            o_sb)
    nc.compile()
    return nc


_cache = {}


def kernel(x, freqs_cis, W_attn, W_proj, _trace=False, _timing=None):
    from concourse.bass_utils import run_bass_kernel_spmd

    per_core, _ = host_prep(x, freqs_cis, W_attn, W_proj)

    if "l1" not in _cache:
        _cache["l1"] = build_launch1()
    if "l2" not in _cache:
        _cache["l2"] = build_launch2()

    kw = dict(trace=True, trace_cores=list(range(NCORES))) if _trace else {}
    res1 = run_bass_kernel_spmd(_cache["l1"], per_core, list(range(NCORES)), **kw)
    yT_full = np.empty((NH, P, T), np.float16)
    for c in range(NCORES):
        yT_full[2 * c:2 * c + 2] = res1.results[c]["yt"]

    pc2 = host_prep_proj(yT_full, W_proj)
    res2 = run_bass_kernel_spmd(_cache["l2"], pc2, list(range(NCORES)), **kw)
    out = np.concatenate([res2.results[c]["out"] for c in range(NCORES)], axis=0)

    if _timing is not None:
        _timing["l1_ns"] = res1.exec_time_ns
        _timing["l2_ns"] = res2.exec_time_ns
        _timing["res1"] = res1
        _timing["res2"] = res2
    return out.reshape(B, T, C)


# revision 11
# speedup vs baseline: 1.2938x; 1.0459x over previous
"""Trainium2 Bass kernel for scrambled-GQA sliding-window attention.

Head-parallel across 8 NeuronCores, two SPMD launches, no collectives:
  launch 1: QKV projection + RoPE + banded attention -> per-core y^T (2 heads)
  launch 2: output projection, sequence-parallel rows -> per-core 512 output rows

All matmul operands are fp16 (1 col/cycle on PE, FWL weight loads, half DMA).
The two heads per core share K/V blocks, so score matmuls pair them as one
1024-column fp16 matmul (one PSUM bank).  Softmax denominators come from a
DVE-accumulated sum of exp tiles plus a single ones-matmul per (ti) tile
instead of one per block.  Q projection is interleaved with attention tiles
(tile ti only needs m-chunks 2ti, 2ti+1) to keep PE dense while ACT runs exp.

The torch-faithful "scrambled" reshapes in the reference are equivalent to
reinterpreting column slices of qkv = x @ W_attn:
  Q^T_h[d, m*256+t''] = qkv[t''*16+h, c_q(m)*128+d],  c_q(m)=m+2*(m//4), m in [0,16)
  K^T_h'[d, g*1024+u] = qkv[u*4+h', (6g+4)*128+d],    g in [0,4)
  V_h'[g*1024+u, d]   = qkv[u*4+h', (6g+5)*128+d]
Head h attends K/V block h' = h//4 over all rows with band |tq-tk| <= 1023.
RoPE applied to Q,K at position = row index (interleaved pairs).
"""

import math

import numpy as np

B, T, C = 1, 4096, 2048
NH, NKV, HD = 16, 4, 128
WINDOW = 1024
NCORES = 8
P = 128
KC = C // P            # 16 contraction chunks
NM = 16                # scramble chunks (m)
SCALE = 1.0 / math.sqrt(HD)

TQ = 512               # tq tile
NT = T // TQ           # 8 tiles per head
NBLK = T // P          # 32 tk blocks


def _cq(m):
    return m + 2 * (m // 4)


def _block_range(ti):
    b0 = 4 * ti
    return max(0, b0 - 8), min(NBLK - 1, b0 + 11)


def _mask_patterns():
    """Partial-band mask tiles keyed by D = tq0 - tkb (multiples of 128)."""
    ds = [640, 768, 896, 1024, -1024, -1152, -1280, -1408]
    tk = np.arange(P)[:, None]
    tq = np.arange(TQ)[None, :]
    masks = {}
    for d in ds:
        masks[d] = (np.abs(d + tq - tk) <= (WINDOW - 1)).astype(np.float32)
    return ds, masks


def host_prep(x, freqs_cis, W_attn, W_proj):
    """Build all per-core / shared numpy inputs for launch 1 (fp16)."""
    x = np.asarray(x, np.float32)
    freqs_cis = np.asarray(freqs_cis, np.float32)
    W_attn = np.asarray(W_attn, np.float32)

    xT = np.ascontiguousarray(x[0].T).astype(np.float16)    # (C, T)

    # RoPE tables, (128, T): rows 2i,2i+1 = cos(ang[:, i]); sin signed.
    cos = np.repeat(freqs_cis[:, :, 0].T, 2, axis=0).astype(np.float16)
    sin_base = freqs_cis[:, :, 1].T                          # (64, T)
    sin = np.empty((P, T), np.float32)
    sin[0::2] = -sin_base
    sin[1::2] = sin_base
    sin = sin.astype(np.float16)

    # pair-swap matrix (symmetric): row 2i <-> row 2i+1
    mt = np.zeros((P, P), np.float16)
    ii = np.arange(0, P, 2)
    mt[ii, ii + 1] = 1.0
    mt[ii + 1, ii] = 1.0

    mask_ds, masks = _mask_patterns()
    masks_arr = np.ascontiguousarray(
        np.stack([masks[d] for d in mask_ds], axis=1)).astype(np.float16)

    wa3 = W_attn.reshape(KC, P, 24, HD)          # [kc][p][blk][d]
    wq = np.stack(
        [np.ascontiguousarray(
            wa3[:, :, _cq(m), :].transpose(1, 0, 2).reshape(P, KC * HD))
         for m in range(NM)]
    ).astype(np.float16)                         # (16, 128, 2048)
    wk = np.stack(
        [np.ascontiguousarray(
            wa3[:, :, 6 * g + 4, :].transpose(1, 0, 2).reshape(P, KC * HD))
         for g in range(NKV)]
    ).astype(np.float16)                         # (4, 128, 2048)
    wv = np.ascontiguousarray(
        np.concatenate([wa3[:, :, 6 * g + 5, :] for g in range(NKV)], axis=2)
        .transpose(1, 0, 2)
        .reshape(P, KC, NKV * HD)
    ).astype(np.float16)                         # (128, 16, 512)

    per_core = []
    for c in range(NCORES):
        hp = c // 2
        cols = np.concatenate([np.arange(256) * 16 + (2 * c + z) for z in (0, 1)])
        xq = np.ascontiguousarray(
            xT[:, cols].reshape(KC, P, 512).transpose(1, 0, 2))   # (128, 16, 512)
        ucols = np.arange(1024) * 4 + hp
        xkv = np.ascontiguousarray(
            xT[:, ucols].reshape(KC, P, 1024).transpose(1, 0, 2))  # (128, 16, 1024)
        per_core.append(
            dict(xq=xq, xkv=xkv, wq=wq, wk=wk, wv=wv, cos=cos, sin=sin,
                 mt=mt, ones=np.ones((P, P), np.float16), masks=masks_arr)
        )
    return per_core, mask_ds


def host_prep_proj(yT_full, W_proj):
    """yT_full: (16, 128, 4096) fp16 per-head transposed attention output."""
    W_proj = np.asarray(W_proj, np.float32)
    wp = np.ascontiguousarray(W_proj.reshape(NH, HD, C)).astype(np.float16)
    per_core = []
    for c in range(NCORES):
        yt = np.ascontiguousarray(yT_full[:, :, c * 512:(c + 1) * 512])
        per_core.append(dict(yt=yt, wp=wp))
    return per_core


# ---------------------------------------------------------------------------
# numpy emulation of the exact device algorithm (validates all index math)
# ---------------------------------------------------------------------------

def emulate(x, freqs_cis, W_attn, W_proj):
    per_core, mask_ds = host_prep(x, freqs_cis, W_attn, W_proj)
    _, masks = _mask_patterns()
    yT_full = np.zeros((NH, P, T), np.float32)
    for c in range(NCORES):
        d = per_core[c]
        xq = d["xq"].astype(np.float32).transpose(1, 0, 2).reshape(C, 512)
        xkv = d["xkv"].astype(np.float32).transpose(1, 0, 2).reshape(C, 1024)
        cos = d["cos"].astype(np.float32)
        sin = d["sin"].astype(np.float32)
        mt = d["mt"].astype(np.float32)
        qr = np.zeros((2, P, T), np.float32)
        for m in range(NM):
            wq_full = d["wq"][m].astype(np.float32).reshape(P, KC, HD)\
                .transpose(1, 0, 2).reshape(C, HD)
            qt = wq_full.T @ xq                               # (128, 512)
            qsw = mt @ qt
            c2 = np.concatenate([cos[:, m * 256:(m + 1) * 256]] * 2, axis=1)
            s2 = np.concatenate([sin[:, m * 256:(m + 1) * 256]] * 2, axis=1)
            qt = qt * c2 + qsw * s2
            qr[0, :, m * 256:(m + 1) * 256] = qt[:, :256]
            qr[1, :, m * 256:(m + 1) * 256] = qt[:, 256:]
        kr = np.zeros((P, T), np.float32)
        for g in range(NKV):
            wkg = d["wk"][g].astype(np.float32).reshape(P, KC, HD)\
                .transpose(1, 0, 2).reshape(C, HD)
            kt = wkg.T @ xkv                                  # (128, 1024)
            ksw = mt @ kt
            sl = slice(g * 1024, (g + 1) * 1024)
            kr[:, sl] = kt * cos[:, sl] + ksw * sin[:, sl]
        vall = np.zeros((P, 8, 512), np.float32)
        wv_full = d["wv"].astype(np.float32).transpose(1, 0, 2).reshape(C, 512)
        for ut in range(8):
            vall[:, ut, :] = xkv[:, ut * 128:(ut + 1) * 128].T @ wv_full
        for z in range(2):
            for ti in range(NT):
                blo, bhi = _block_range(ti)
                q_tile = qr[z, :, ti * TQ:(ti + 1) * TQ]
                y_acc = np.zeros((P, TQ), np.float32)
                s_acc = np.zeros((TQ,), np.float32)
                for b in range(blo, bhi + 1):
                    st = kr[:, b * P:(b + 1) * P].T @ q_tile  # (128tk, 512)
                    pt = np.exp(SCALE * st)
                    D = 512 * ti - 128 * b
                    if D in masks:
                        pt = pt * masks[D]
                    g, ub = b // 8, b % 8
                    vblk = vall[:, ub, g * HD:(g + 1) * HD]   # (128u, 128d)
                    y_acc += vblk.T @ pt
                    s_acc += pt.sum(axis=0)
                yT_full[2 * c + z, :, ti * TQ:(ti + 1) * TQ] = y_acc / s_acc[None, :]
    pc = host_prep_proj(yT_full.astype(np.float16), W_proj)
    outs = []
    for c in range(NCORES):
        yt = pc[c]["yt"].astype(np.float32)
        wp = pc[c]["wp"].astype(np.float32)
        acc = np.zeros((512, C), np.float32)
        for h in range(NH):
            acc += yt[h].T @ wp[h]
        outs.append(acc)
    return np.concatenate(outs, axis=0).reshape(B, T, C)


# ---------------------------------------------------------------------------
# Bass programs
# ---------------------------------------------------------------------------

def build_launch1():
    import concourse.bacc as bacc
    import concourse.mybir as mybir
    import concourse.tile as tile

    import concourse.bass_isa as bass_isa

    f16 = mybir.dt.float16
    f32 = mybir.dt.float32
    MUL = mybir.AluOpType.mult
    ADD = mybir.AluOpType.add
    EXP = mybir.ActivationFunctionType.Exp

    nc = bacc.Bacc("TRN2", target_bir_lowering=False, debug=False)

    xq_d = nc.dram_tensor("xq", (P, KC, 512), f16, kind="ExternalInput")
    xkv_d = nc.dram_tensor("xkv", (P, KC, 1024), f16, kind="ExternalInput")
    wq_d = nc.dram_tensor("wq", (NM, P, KC * HD), f16, kind="ExternalInput")
    wk_d = nc.dram_tensor("wk", (NKV, P, KC * HD), f16, kind="ExternalInput")
    wv_d = nc.dram_tensor("wv", (P, KC, 512), f16, kind="ExternalInput")
    cos_d = nc.dram_tensor("cos", (P, T), f16, kind="ExternalInput")
    sin_d = nc.dram_tensor("sin", (P, T), f16, kind="ExternalInput")
    mt_d = nc.dram_tensor("mt", (P, P), f16, kind="ExternalInput")
    ones_d = nc.dram_tensor("ones", (P, P), f16, kind="ExternalInput")
    masks_d = nc.dram_tensor("masks", (P, 8, TQ), f16, kind="ExternalInput")
    yt_d = nc.dram_tensor("yt", (2, P, T), f16, kind="ExternalOutput")

    mask_ds, _ = _mask_patterns()
    mask_idx = {d: i for i, d in enumerate(mask_ds)}

    with tile.TileContext(nc) as tc:
        with tc.tile_pool(name="persist", bufs=1) as persist:
            qr = persist.tile([P, NT, 2, TQ], f16, tag="qr", name="qr")
            kr = persist.tile([P, T], f16, tag="kr", name="kr")
            vall = persist.tile([P, 8, TQ], f16, tag="vall", name="vall")
            ones = persist.tile([P, P], f16, tag="ones", name="ones")
            mt_s = persist.tile([P, P], f16, tag="mt", name="mt_s")
            nc.sync.dma_start(ones[:], ones_d.ap())
            masks_s = persist.tile([P, 8, TQ], f16, tag="masks", name="masks_s")
            xq_s = persist.tile([P, KC, 512], f16, tag="xq", name="xq_s")
            xkv_s = persist.tile([P, KC, 1024], f16, tag="xkv", name="xkv_s")
            nc.sync.dma_start(mt_s[:], mt_d.ap())
            nc.sync.dma_start(masks_s[:], masks_d.ap())

            with tc.tile_pool(name="wstream", bufs=2) as wsp:

                # ---- V (kc-outer: PE starts after first small DMA) ----
                with tc.tile_pool(name="vps", bufs=8, space="PSUM") as vps:
                    vpss = [vps.tile([P, TQ], f32, tag="vpsum",
                                     name="vpsum") for _ in range(8)]
                    for kc in range(KC):
                        nc.sync.dma_start(xkv_s[:, kc], xkv_d.ap()[:, kc])
                        wv_c = wsp.tile([P, 1, 512], f16, tag="wv",
                                        bufs=3, name="wv_c")
                        nc.sync.dma_start(wv_c[:, 0], wv_d.ap()[:, kc])
                        for ut in range(8):
                            nc.tensor.matmul(
                                vpss[ut],
                                xkv_s[:, kc, ut * P:(ut + 1) * P],
                                wv_c[:, 0],
                                start=(kc == 0), stop=(kc == KC - 1))
                    for ut in range(8):
                        nc.scalar.copy(vall[:, ut], vpss[ut])

                # prefetch wk while V finishes; xq after
                wk_ss = []
                for g in range(NKV):
                    wk_s = wsp.tile([P, KC, HD], f16, tag="wk", bufs=4,
                                    name="wk_s")
                    nc.sync.dma_start(
                        wk_s[:],
                        wk_d.ap()[g].rearrange("p (kc d) -> p kc d", d=HD))
                    wk_ss.append(wk_s)
                for kc in range(KC):
                    nc.sync.dma_start(xq_s[:, kc], xq_d.ap()[:, kc])

                def rope(ppool, sw_tag, sw_bufs, sb, tsl, n, out_ap, view, vtab):
                    """out = sb*cos + (M @ sb)*sin; sb is flat (128, 512) SBUF."""
                    ctab = wsp.tile([P, TQ], f16, tag="ctab", bufs=2, name="ctab")
                    nc.sync.dma_start(ctab[:, :n], cos_d.ap()[:, tsl])
                    stab = wsp.tile([P, TQ], f16, tag="stab", bufs=2, name="stab")
                    nc.sync.dma_start(stab[:, :n], sin_d.ap()[:, tsl])
                    sw = ppool.tile([P, TQ], f32, tag=sw_tag, bufs=sw_bufs,
                                    name=sw_tag)
                    nc.tensor.matmul(sw, mt_s[:], sb, start=True, stop=True)
                    t1 = wsp.tile([P, TQ], f16, tag="rope_t1", bufs=2,
                                  name="rope_t1")
                    nc.vector.tensor_tensor(view(t1), view(sb), vtab(ctab), MUL)
                    t2 = wsp.tile([P, TQ], f16, tag="rope_t2", bufs=2,
                                  name="rope_t2")
                    nc.vector.tensor_tensor(view(t2), view(sw), vtab(stab), MUL)
                    nc.vector.tensor_tensor(out_ap, view(t1), view(t2), ADD)

                # ---- K ----
                with tc.tile_pool(name="kqp", bufs=1, space="PSUM") as kqps:
                    for g in range(NKV):
                        for ut in range(2):
                            ps = kqps.tile([P, TQ], f32, tag="kps", bufs=2,
                                           name="kpsum")
                            for kc in range(KC):
                                nc.tensor.matmul(
                                    ps, wk_ss[g][:, kc],
                                    xkv_s[:, kc, ut * TQ:(ut + 1) * TQ],
                                    start=(kc == 0), stop=(kc == KC - 1))
                            ksb = wsp.tile([P, TQ], f16, tag="sbr", bufs=2,
                                           name="ksb")
                            nc.scalar.copy(ksb, ps)
                            sl = slice(g * 1024 + ut * TQ,
                                       g * 1024 + (ut + 1) * TQ)
                            rope(kqps, "ksw", 2, ksb, sl, TQ, kr[:, sl],
                                 lambda a: a, lambda tb: tb[:, :TQ])

                # ---- interleaved Q projection + attention ----
                # PSUM banks: big(2x2) + y(2) + qsw(1) + sq(1) = 8
                with tc.tile_pool(name="aps", bufs=1, space="PSUM") as aps, \
                     tc.tile_pool(name="asb", bufs=2) as asb:
                    for ti in range(NT):
                        for mh in range(2):
                            m = 2 * ti + mh
                            wq_s = wsp.tile([P, KC, HD], f16, tag="wq", bufs=3,
                                            name="wq_s")
                            nc.sync.dma_start(
                                wq_s[:],
                                wq_d.ap()[m].rearrange("p (kc d) -> p kc d",
                                                       d=HD))
                            qps = aps.tile([P, 2, TQ], f32, tag="big", bufs=2,
                                           name="qpsum")
                            for kc in range(KC):
                                nc.tensor.matmul(
                                    qps[:, 0], wq_s[:, kc], xq_s[:, kc],
                                    start=(kc == 0), stop=(kc == KC - 1))
                            qsb = wsp.tile([P, TQ], f16, tag="sbr", bufs=2,
                                           name="qsb")
                            nc.vector.tensor_copy(qsb, qps[:, 0])
                            rope(aps, "qsw", 1, qsb,
                                 slice(m * 256, (m + 1) * 256), 256,
                                 qr[:, ti, :, mh * 256:mh * 256 + 256],
                                 lambda a: a.rearrange("p (z t) -> p z t", z=2),
                                 lambda tb: tb[:, None, :256].broadcast_to(
                                     (P, 2, 256)))

                        blo, bhi = _block_range(ti)
                        bs = (list(range(4 * ti, bhi + 1))
                              + list(range(blo, 4 * ti)))
                        nb = len(bs)
                        q_rhs = qr[:, ti]                     # (128, 2, 512)
                        y_ps = aps.tile([P, 2, TQ], f32, tag="y", bufs=1,
                                        name="y_ps")
                        acc = asb.tile([P, 2, TQ], f16, tag="acc", bufs=2,
                                       name="acc")
                        nc.vector.memset(acc[:], 0.0)
                        for j, b in enumerate(bs):
                            D = 512 * ti - 128 * b
                            # in-band tq range: |D + tq - tk| <= 1023
                            lo = max(0, -1023 - D)
                            hi = min(TQ, 1151 - D)
                            st = aps.tile([P, 2, TQ], f32, tag="big", bufs=2,
                                          name="st_ps")
                            for z in range(2):
                                nc.tensor.matmul(
                                    st[:, z, lo:hi], kr[:, b * P:(b + 1) * P],
                                    q_rhs[:, z, lo:hi], start=True, stop=True)
                            pt = asb.tile([P, 2, TQ], f16, tag="pt", bufs=3,
                                          name="pt")
                            nc.scalar.activation(pt[:, :, lo:hi],
                                                 st[:, :, lo:hi], EXP,
                                                 scale=SCALE)
                            if D in mask_idx:
                                nc.vector.tensor_tensor(
                                    pt[:, :, lo:hi], pt[:, :, lo:hi],
                                    masks_s[:, mask_idx[D]][:, None, lo:hi]
                                    .broadcast_to((P, 2, hi - lo)),
                                    MUL)
                            nc.vector.tensor_tensor(
                                acc[:, :, lo:hi], acc[:, :, lo:hi],
                                pt[:, :, lo:hi], ADD)
                            g, ub = b // 8, b % 8
                            for z in range(2):
                                nc.tensor.matmul(
                                    y_ps[:, z, lo:hi],
                                    vall[:, ub, g * HD:(g + 1) * HD],
                                    pt[:, z, lo:hi],
                                    start=(j == 0), stop=(j == nb - 1))
                        r_sb = asb.tile([1, 2, TQ], f32, tag="rsb", bufs=2,
                                        name="r_sb")
                        rb_sb = asb.tile([P, 2, TQ], f32, tag="rb", bufs=2,
                                         name="rb_sb")
                        for z in range(2):
                            sq = aps.tile([1, TQ], f32, tag="sq", bufs=1,
                                          name="sq")
                            nc.tensor.matmul(sq, ones[:, 0:1], acc[:, z],
                                             start=True, stop=True)
                            nc.vector.reciprocal_approx_fast(r_sb[:, z], sq)
                            nc.gpsimd.partition_broadcast(rb_sb[:, z],
                                                          r_sb[:, z])
                        y_sb = asb.tile([P, 2, TQ], f16, tag="ysb", bufs=2,
                                        name="y_sb")
                        nc.vector.tensor_tensor(y_sb, y_ps, rb_sb, MUL)
                        for z in range(2):
                            nc.sync.dma_start(
                                yt_d.ap()[z, :, ti * TQ:(ti + 1) * TQ],
                                y_sb[:, z])

    nc.compile()
    return nc


def build_launch2():
    import concourse.bacc as bacc
    import concourse.mybir as mybir
    import concourse.tile as tile

    f16 = mybir.dt.float16
    f32 = mybir.dt.float32

    nc = bacc.Bacc("TRN2", target_bir_lowering=False, debug=False)
    yt_d = nc.dram_tensor("yt", (NH, P, 512), f16, kind="ExternalInput")
    wp_d = nc.dram_tensor("wp", (NH, P, C), f16, kind="ExternalInput")
    out_d = nc.dram_tensor("out", (512, C), f32, kind="ExternalOutput")

    with tile.TileContext(nc) as tc:
        with tc.tile_pool(name="sb", bufs=2) as sb, \
             tc.tile_pool(name="ps", bufs=8, space="PSUM") as psp:
            yt_s = sb.tile([P, NH, 512], f16, tag="yt", bufs=1, name="yt_s")
            wp_s = sb.tile([P, NH, C], f16, tag="wp", bufs=1, name="wp_s")
            for h in range(NH):
                nc.sync.dma_start(yt_s[:, h], yt_d.ap()[h])
                nc.sync.dma_start(wp_s[:, h], wp_d.ap()[h])
            # h-outer accumulation: 8 resident psum tiles per ct-half so the
            # first matmuls only wait on wp[0]/yt[0]; lhsT shared across ct.
            for ch in range(2):
                pss = [[psp.tile([P, 512], f32, tag="ps", name="ps")
                        for _ in range(2)] for _ in range(4)]
                for h in range(NH):
                    for tt in range(4):
                        for c2 in range(2):
                            ct = ch * 2 + c2
                            nc.tensor.matmul(
                                pss[tt][c2], yt_s[:, h, tt * P:(tt + 1) * P],
                                wp_s[:, h, ct * 512:(ct + 1) * 512],
                                start=(h == 0), stop=(h == NH - 1))
                for tt in range(4):
                    for c2 in range(2):
                        ct = ch * 2 + c2
                        o_sb = sb.tile([P, 512], f32, tag="osb", bufs=4,
                                       name="o_sb")
                        nc.vector.tensor_copy(o_sb, pss[tt][c2])
                        nc.sync.dma_start(
                            out_d.ap()[tt * P:(tt + 1) * P,
                                       ct * 512:(ct + 1) * 512],
                            o_sb)
    nc.compile()
    return nc


_cache = {}


def kernel(x, freqs_cis, W_attn, W_proj, _trace=False, _timing=None):
    from concourse.bass_utils import run_bass_kernel_spmd

    per_core, _ = host_prep(x, freqs_cis, W_attn, W_proj)

    if "l1" not in _cache:
        _cache["l1"] = build_launch1()
    if "l2" not in _cache:
        _cache["l2"] = build_launch2()

    kw = dict(trace=True, trace_cores=list(range(NCORES))) if _trace else {}
    res1 = run_bass_kernel_spmd(_cache["l1"], per_core, list(range(NCORES)), **kw)
    yT_full = np.empty((NH, P, T), np.float16)
    for c in range(NCORES):
        yT_full[2 * c:2 * c + 2] = res1.results[c]["yt"]

    pc2 = host_prep_proj(yT_full, W_proj)
    res2 = run_bass_kernel_spmd(_cache["l2"], pc2, list(range(NCORES)), **kw)
    out = np.concatenate([res2.results[c]["out"] for c in range(NCORES)], axis=0)

    if _timing is not None:
        _timing["l1_ns"] = res1.exec_time_ns
        _timing["l2_ns"] = res2.exec_time_ns
        _timing["res1"] = res1
        _timing["res2"] = res2
    return out.reshape(B, T, C)


# revision 14
# speedup vs baseline: 1.3262x; 1.0251x over previous
"""Trainium2 Bass kernel for scrambled-GQA sliding-window attention.

Head-parallel across 8 NeuronCores, two SPMD launches, no collectives:
  launch 1: QKV projection + RoPE + banded attention -> per-core y^T (2 heads)
  launch 2: output projection, sequence-parallel rows -> per-core 512 output rows

All matmul operands are fp16 (1 col/cycle on PE, FWL weight loads, half DMA).
The two heads per core share K/V blocks, so score matmuls pair them as one
1024-column fp16 matmul (one PSUM bank).  Softmax denominators come from a
DVE-accumulated sum of exp tiles plus a single ones-matmul per (ti) tile
instead of one per block.  Q projection is interleaved with attention tiles
(tile ti only needs m-chunks 2ti, 2ti+1) to keep PE dense while ACT runs exp.

The torch-faithful "scrambled" reshapes in the reference are equivalent to
reinterpreting column slices of qkv = x @ W_attn:
  Q^T_h[d, m*256+t''] = qkv[t''*16+h, c_q(m)*128+d],  c_q(m)=m+2*(m//4), m in [0,16)
  K^T_h'[d, g*1024+u] = qkv[u*4+h', (6g+4)*128+d],    g in [0,4)
  V_h'[g*1024+u, d]   = qkv[u*4+h', (6g+5)*128+d]
Head h attends K/V block h' = h//4 over all rows with band |tq-tk| <= 1023.
RoPE applied to Q,K at position = row index (interleaved pairs).
"""

import math

import numpy as np

B, T, C = 1, 4096, 2048
NH, NKV, HD = 16, 4, 128
WINDOW = 1024
NCORES = 8
P = 128
KC = C // P            # 16 contraction chunks
NM = 16                # scramble chunks (m)
SCALE = 1.0 / math.sqrt(HD)

TQ = 512               # tq tile
NT = T // TQ           # 8 tiles per head
NBLK = T // P          # 32 tk blocks


def _cq(m):
    return m + 2 * (m // 4)


def _block_range(ti):
    b0 = 4 * ti
    return max(0, b0 - 8), min(NBLK - 1, b0 + 11)


def _mask_patterns():
    """Partial-band mask tiles keyed by D = tq0 - tkb (multiples of 128)."""
    ds = [640, 768, 896, 1024, -1024, -1152, -1280, -1408]
    tk = np.arange(P)[:, None]
    tq = np.arange(TQ)[None, :]
    masks = {}
    for d in ds:
        masks[d] = (np.abs(d + tq - tk) <= (WINDOW - 1)).astype(np.float32)
    return ds, masks


def host_prep(x, freqs_cis, W_attn, W_proj):
    """Build all per-core / shared numpy inputs for launch 1 (fp16)."""
    x = np.asarray(x, np.float32)
    freqs_cis = np.asarray(freqs_cis, np.float32)
    W_attn = np.asarray(W_attn, np.float32)

    xT = np.ascontiguousarray(x[0].T).astype(np.float16)    # (C, T)

    # RoPE tables, (128, T): rows 2i,2i+1 = cos(ang[:, i]); sin signed.
    cos = np.repeat(freqs_cis[:, :, 0].T, 2, axis=0).astype(np.float16)
    sin_base = freqs_cis[:, :, 1].T                          # (64, T)
    sin = np.empty((P, T), np.float32)
    sin[0::2] = -sin_base
    sin[1::2] = sin_base
    sin = sin.astype(np.float16)

    # pair-swap matrix (symmetric): row 2i <-> row 2i+1
    mt = np.zeros((P, P), np.float16)
    ii = np.arange(0, P, 2)
    mt[ii, ii + 1] = 1.0
    mt[ii + 1, ii] = 1.0

    mask_ds, masks = _mask_patterns()
    masks_arr = np.ascontiguousarray(
        np.stack([masks[d] for d in mask_ds], axis=1)).astype(np.float16)

    wa3 = W_attn.reshape(KC, P, 24, HD)          # [kc][p][blk][d]
    wq = np.stack(
        [np.ascontiguousarray(
            wa3[:, :, _cq(m), :].transpose(1, 0, 2).reshape(P, KC * HD))
         for m in range(NM)]
    ).astype(np.float16)                         # (16, 128, 2048)
    wk = np.stack(
        [np.ascontiguousarray(
            wa3[:, :, 6 * g + 4, :].transpose(1, 0, 2).reshape(P, KC * HD))
         for g in range(NKV)]
    ).astype(np.float16)                         # (4, 128, 2048)
    wv = np.ascontiguousarray(
        np.concatenate([wa3[:, :, 6 * g + 5, :] for g in range(NKV)], axis=2)
        .transpose(1, 0, 2)
        .reshape(P, KC, NKV * HD)
    ).astype(np.float16)                         # (128, 16, 512)

    per_core = []
    for c in range(NCORES):
        hp = c // 2
        cols = np.concatenate([np.arange(256) * 16 + (2 * c + z) for z in (0, 1)])
        xq = np.ascontiguousarray(
            xT[:, cols].reshape(KC, P, 512).transpose(1, 0, 2))   # (128, 16, 512)
        ucols = np.arange(1024) * 4 + hp
        xkv = np.ascontiguousarray(
            xT[:, ucols].reshape(KC, P, 1024).transpose(1, 0, 2))  # (128, 16, 1024)
        per_core.append(
            dict(xq=xq, xkv=xkv, wq=wq, wk=wk, wv=wv, cos=cos, sin=sin,
                 mt=mt, ones=np.ones((P, P), np.float16), masks=masks_arr)
        )
    return per_core, mask_ds


def host_prep_proj(yT_full, W_proj):
    """yT_full: (16, 128, 4096) fp16 per-head transposed attention output."""
    W_proj = np.asarray(W_proj, np.float32)
    wp = np.ascontiguousarray(W_proj.reshape(NH, HD, C)).astype(np.float16)
    per_core = []
    for c in range(NCORES):
        yt = np.ascontiguousarray(yT_full[:, :, c * 512:(c + 1) * 512])
        per_core.append(dict(yt=yt, wp=wp))
    return per_core


# ---------------------------------------------------------------------------
# numpy emulation of the exact device algorithm (validates all index math)
# ---------------------------------------------------------------------------

def emulate(x, freqs_cis, W_attn, W_proj):
    per_core, mask_ds = host_prep(x, freqs_cis, W_attn, W_proj)
    _, masks = _mask_patterns()
    yT_full = np.zeros((NH, P, T), np.float32)
    for c in range(NCORES):
        d = per_core[c]
        xq = d["xq"].astype(np.float32).transpose(1, 0, 2).reshape(C, 512)
        xkv = d["xkv"].astype(np.float32).transpose(1, 0, 2).reshape(C, 1024)
        cos = d["cos"].astype(np.float32)
        sin = d["sin"].astype(np.float32)
        mt = d["mt"].astype(np.float32)
        qr = np.zeros((2, P, T), np.float32)
        for m in range(NM):
            wq_full = d["wq"][m].astype(np.float32).reshape(P, KC, HD)\
                .transpose(1, 0, 2).reshape(C, HD)
            qt = wq_full.T @ xq                               # (128, 512)
            qsw = mt @ qt
            c2 = np.concatenate([cos[:, m * 256:(m + 1) * 256]] * 2, axis=1)
            s2 = np.concatenate([sin[:, m * 256:(m + 1) * 256]] * 2, axis=1)
            qt = qt * c2 + qsw * s2
            qr[0, :, m * 256:(m + 1) * 256] = qt[:, :256]
            qr[1, :, m * 256:(m + 1) * 256] = qt[:, 256:]
        kr = np.zeros((P, T), np.float32)
        for g in range(NKV):
            wkg = d["wk"][g].astype(np.float32).reshape(P, KC, HD)\
                .transpose(1, 0, 2).reshape(C, HD)
            kt = wkg.T @ xkv                                  # (128, 1024)
            ksw = mt @ kt
            sl = slice(g * 1024, (g + 1) * 1024)
            kr[:, sl] = kt * cos[:, sl] + ksw * sin[:, sl]
        vall = np.zeros((P, 8, 512), np.float32)
        wv_full = d["wv"].astype(np.float32).transpose(1, 0, 2).reshape(C, 512)
        for ut in range(8):
            vall[:, ut, :] = xkv[:, ut * 128:(ut + 1) * 128].T @ wv_full
        for z in range(2):
            for ti in range(NT):
                blo, bhi = _block_range(ti)
                q_tile = qr[z, :, ti * TQ:(ti + 1) * TQ]
                y_acc = np.zeros((P, TQ), np.float32)
                s_acc = np.zeros((TQ,), np.float32)
                for b in range(blo, bhi + 1):
                    st = kr[:, b * P:(b + 1) * P].T @ q_tile  # (128tk, 512)
                    pt = np.exp(SCALE * st)
                    D = 512 * ti - 128 * b
                    if D in masks:
                        pt = pt * masks[D]
                    g, ub = b // 8, b % 8
                    vblk = vall[:, ub, g * HD:(g + 1) * HD]   # (128u, 128d)
                    y_acc += vblk.T @ pt
                    s_acc += pt.sum(axis=0)
                yT_full[2 * c + z, :, ti * TQ:(ti + 1) * TQ] = y_acc / s_acc[None, :]
    pc = host_prep_proj(yT_full.astype(np.float16), W_proj)
    outs = []
    for c in range(NCORES):
        yt = pc[c]["yt"].astype(np.float32)
        wp = pc[c]["wp"].astype(np.float32)
        acc = np.zeros((512, C), np.float32)
        for h in range(NH):
            acc += yt[h].T @ wp[h]
        outs.append(acc)
    return np.concatenate(outs, axis=0).reshape(B, T, C)


# ---------------------------------------------------------------------------
# Bass programs
# ---------------------------------------------------------------------------

def build_launch1():
    import concourse.bacc as bacc
    import concourse.mybir as mybir
    import concourse.tile as tile

    import concourse.bass_isa as bass_isa

    f16 = mybir.dt.float16
    f32 = mybir.dt.float32
    MUL = mybir.AluOpType.mult
    ADD = mybir.AluOpType.add
    EXP = mybir.ActivationFunctionType.Exp

    nc = bacc.Bacc("TRN2", target_bir_lowering=False, debug=False)

    xq_d = nc.dram_tensor("xq", (P, KC, 512), f16, kind="ExternalInput")
    xkv_d = nc.dram_tensor("xkv", (P, KC, 1024), f16, kind="ExternalInput")
    wq_d = nc.dram_tensor("wq", (NM, P, KC * HD), f16, kind="ExternalInput")
    wk_d = nc.dram_tensor("wk", (NKV, P, KC * HD), f16, kind="ExternalInput")
    wv_d = nc.dram_tensor("wv", (P, KC, 512), f16, kind="ExternalInput")
    cos_d = nc.dram_tensor("cos", (P, T), f16, kind="ExternalInput")
    sin_d = nc.dram_tensor("sin", (P, T), f16, kind="ExternalInput")
    mt_d = nc.dram_tensor("mt", (P, P), f16, kind="ExternalInput")
    ones_d = nc.dram_tensor("ones", (P, P), f16, kind="ExternalInput")
    masks_d = nc.dram_tensor("masks", (P, 8, TQ), f16, kind="ExternalInput")
    yt_d = nc.dram_tensor("yt", (2, P, T), f16, kind="ExternalOutput")

    mask_ds, _ = _mask_patterns()
    mask_idx = {d: i for i, d in enumerate(mask_ds)}

    with tile.TileContext(nc) as tc:
        with tc.tile_pool(name="persist", bufs=1) as persist:
            qr = persist.tile([P, NT, 2, TQ], f16, tag="qr", name="qr")
            kr = persist.tile([P, T], f16, tag="kr", name="kr")
            vall = persist.tile([P, 8, TQ], f16, tag="vall", name="vall")
            ones = persist.tile([P, P], f16, tag="ones", name="ones")
            mt_s = persist.tile([P, P], f16, tag="mt", name="mt_s")
            masks_s = persist.tile([P, 8, TQ], f16, tag="masks", name="masks_s")
            xq_s = persist.tile([P, KC, 512], f16, tag="xq", name="xq_s")
            xkv_s = persist.tile([P, KC, 1024], f16, tag="xkv", name="xkv_s")

            with tc.tile_pool(name="wstream", bufs=2) as wsp:

                # ---- V (kc-outer: PE starts after first small DMA) ----
                with tc.tile_pool(name="vps", bufs=8, space="PSUM") as vps:
                    vpss = [vps.tile([P, TQ], f32, tag="vpsum",
                                     name="vpsum") for _ in range(8)]
                    for kc in range(KC):
                        nc.sync.dma_start(xkv_s[:, kc], xkv_d.ap()[:, kc])
                        wv_c = wsp.tile([P, 1, 512], f16, tag="wv",
                                        bufs=3, name="wv_c")
                        nc.sync.dma_start(wv_c[:, 0], wv_d.ap()[:, kc])
                        for ut in range(8):
                            nc.tensor.matmul(
                                vpss[ut],
                                xkv_s[:, kc, ut * P:(ut + 1) * P],
                                wv_c[:, 0],
                                start=(kc == 0), stop=(kc == KC - 1))
                    for ut in range(8):
                        nc.scalar.copy(vall[:, ut], vpss[ut])

                # prefetch wk while V finishes; xq / mt / masks after
                wk_ss = []
                for g in range(NKV):
                    wk_s = wsp.tile([P, KC, HD], f16, tag="wk", bufs=4,
                                    name="wk_s")
                    nc.sync.dma_start(
                        wk_s[:],
                        wk_d.ap()[g].rearrange("p (kc d) -> p kc d", d=HD))
                    wk_ss.append(wk_s)
                nc.sync.dma_start(mt_s[:], mt_d.ap())
                for kc in range(KC):
                    nc.sync.dma_start(xq_s[:, kc], xq_d.ap()[:, kc])
                nc.sync.dma_start(ones[:], ones_d.ap())
                nc.sync.dma_start(masks_s[:], masks_d.ap())

                def rope(ppool, sw_tag, sw_bufs, sb, tsl, n, out_ap, view, vtab):
                    """out = sb*cos + (M @ sb)*sin; sb is flat (128, 512) SBUF."""
                    ctab = wsp.tile([P, TQ], f16, tag="ctab", bufs=2, name="ctab")
                    nc.sync.dma_start(ctab[:, :n], cos_d.ap()[:, tsl])
                    stab = wsp.tile([P, TQ], f16, tag="stab", bufs=2, name="stab")
                    nc.sync.dma_start(stab[:, :n], sin_d.ap()[:, tsl])
                    sw = ppool.tile([P, TQ], f32, tag=sw_tag, bufs=sw_bufs,
                                    name=sw_tag)
                    nc.tensor.matmul(sw, mt_s[:], sb, start=True, stop=True)
                    t1 = wsp.tile([P, TQ], f16, tag="rope_t1", bufs=2,
                                  name="rope_t1")
                    nc.vector.tensor_tensor(view(t1), view(sb), vtab(ctab), MUL)
                    t2 = wsp.tile([P, TQ], f16, tag="rope_t2", bufs=2,
                                  name="rope_t2")
                    nc.vector.tensor_tensor(view(t2), view(sw), vtab(stab), MUL)
                    nc.vector.tensor_tensor(out_ap, view(t1), view(t2), ADD)

                # ---- K ----
                with tc.tile_pool(name="kqp", bufs=1, space="PSUM") as kqps:
                    for g in range(NKV):
                        for ut in range(2):
                            ps = kqps.tile([P, TQ], f32, tag="kps", bufs=2,
                                           name="kpsum")
                            for kc in range(KC):
                                nc.tensor.matmul(
                                    ps, wk_ss[g][:, kc],
                                    xkv_s[:, kc, ut * TQ:(ut + 1) * TQ],
                                    start=(kc == 0), stop=(kc == KC - 1))
                            ksb = wsp.tile([P, TQ], f16, tag="sbr", bufs=2,
                                           name="ksb")
                            nc.scalar.copy(ksb, ps)
                            sl = slice(g * 1024 + ut * TQ,
                                       g * 1024 + (ut + 1) * TQ)
                            rope(kqps, "ksw", 2, ksb, sl, TQ, kr[:, sl],
                                 lambda a: a, lambda tb: tb[:, :TQ])

                # ---- interleaved Q projection + attention ----
                # PSUM banks: big(2x2) + y(2) + qsw(1) + sq(1) = 8
                # Q for tile ti+1 is emitted mid-way through attention(ti) so
                # its rope (DVE) hides under attention matmuls.
                with tc.tile_pool(name="aps", bufs=1, space="PSUM") as aps, \
                     tc.tile_pool(name="asb", bufs=2) as asb:

                    def qproj(m):
                        wq_s = wsp.tile([P, KC, HD], f16, tag="wq", bufs=3,
                                        name="wq_s")
                        nc.sync.dma_start(
                            wq_s[:],
                            wq_d.ap()[m].rearrange("p (kc d) -> p kc d",
                                                   d=HD))
                        qps = aps.tile([P, 2, TQ], f32, tag="big", bufs=2,
                                       name="qpsum")
                        for kc in range(KC):
                            nc.tensor.matmul(
                                qps[:, 0], wq_s[:, kc], xq_s[:, kc],
                                start=(kc == 0), stop=(kc == KC - 1))
                        qsb = wsp.tile([P, TQ], f16, tag="sbr", bufs=2,
                                       name="qsb")
                        nc.vector.tensor_copy(qsb, qps[:, 0])
                        ti, mh = m // 2, m % 2
                        rope(aps, "qsw", 1, qsb,
                             slice(m * 256, (m + 1) * 256), 256,
                             qr[:, ti, :, mh * 256:mh * 256 + 256],
                             lambda a: a.rearrange("p (z t) -> p z t", z=2),
                             lambda tb: tb[:, None, :256].broadcast_to(
                                 (P, 2, 256)))

                    qproj(0)
                    qproj(1)
                    for ti in range(NT):
                        blo, bhi = _block_range(ti)
                        bs = (list(range(4 * ti, bhi + 1))
                              + list(range(blo, 4 * ti)))
                        nb = len(bs)
                        q_rhs = qr[:, ti]                     # (128, 2, 512)
                        y_ps = aps.tile([P, 2, TQ], f32, tag="y", bufs=1,
                                        name="y_ps")
                        acc = asb.tile([P, 2, TQ], f16, tag="acc", bufs=2,
                                       name="acc")
                        nc.gpsimd.memset(acc[:], 0.0)
                        for j, b in enumerate(bs):
                            if ti < NT - 1 and j in (4, 8):
                                qproj(2 * ti + 2 + (j == 8))
                            D = 512 * ti - 128 * b
                            # in-band tq range: |D + tq - tk| <= 1023
                            lo = max(0, -1023 - D)
                            hi = min(TQ, 1151 - D)
                            st = aps.tile([P, 2, TQ], f32, tag="big", bufs=2,
                                          name="st_ps")
                            for z in range(2):
                                nc.tensor.matmul(
                                    st[:, z, lo:hi], kr[:, b * P:(b + 1) * P],
                                    q_rhs[:, z, lo:hi], start=True, stop=True)
                            pt = asb.tile([P, 2, TQ], f16, tag="pt", bufs=3,
                                          name="pt")
                            nc.scalar.activation(pt[:, :, lo:hi],
                                                 st[:, :, lo:hi], EXP,
                                                 scale=SCALE)
                            if D in mask_idx:
                                nc.vector.tensor_tensor(
                                    pt[:, :, lo:hi], pt[:, :, lo:hi],
                                    masks_s[:, mask_idx[D]][:, None, lo:hi]
                                    .broadcast_to((P, 2, hi - lo)),
                                    MUL)
                            nc.vector.tensor_tensor(
                                acc[:, :, lo:hi], acc[:, :, lo:hi],
                                pt[:, :, lo:hi], ADD)
                            g, ub = b // 8, b % 8
                            for z in range(2):
                                nc.tensor.matmul(
                                    y_ps[:, z, lo:hi],
                                    vall[:, ub, g * HD:(g + 1) * HD],
                                    pt[:, z, lo:hi],
                                    start=(j == 0), stop=(j == nb - 1))
                        r_sb = asb.tile([1, 2, TQ], f32, tag="rsb", bufs=2,
                                        name="r_sb")
                        rb_sb = asb.tile([P, 2, TQ], f32, tag="rb", bufs=2,
                                         name="rb_sb")
                        for z in range(2):
                            sq = aps.tile([1, TQ], f32, tag="sq", bufs=1,
                                          name="sq")
                            nc.tensor.matmul(sq, ones[:, 0:1], acc[:, z],
                                             start=True, stop=True)
                            nc.vector.reciprocal_approx_fast(r_sb[:, z], sq)
                            nc.gpsimd.partition_broadcast(rb_sb[:, z],
                                                          r_sb[:, z])
                        y_sb = asb.tile([P, 2, TQ], f16, tag="ysb", bufs=2,
                                        name="y_sb")
                        nc.vector.tensor_tensor(y_sb, y_ps, rb_sb, MUL)
                        for z in range(2):
                            nc.sync.dma_start(
                                yt_d.ap()[z, :, ti * TQ:(ti + 1) * TQ],
                                y_sb[:, z])

    nc.compile()
    return nc


def build_launch2():
    import concourse.bacc as bacc
    import concourse.mybir as mybir
    import concourse.tile as tile

    f16 = mybir.dt.float16
    f32 = mybir.dt.float32

    nc = bacc.Bacc("TRN2", target_bir_lowering=False, debug=False)
    yt_d = nc.dram_tensor("yt", (NH, P, 512), f16, kind="ExternalInput")
    wp_d = nc.dram_tensor("wp", (NH, P, C), f16, kind="ExternalInput")
    out_d = nc.dram_tensor("out", (512, C), f32, kind="ExternalOutput")

    with tile.TileContext(nc) as tc:
        with tc.tile_pool(name="sb", bufs=2) as sb, \
             tc.tile_pool(name="ps", bufs=8, space="PSUM") as psp:
            yt_s = sb.tile([P, NH, 512], f16, tag="yt", bufs=1, name="yt_s")
            wp_s = sb.tile([P, NH, C], f16, tag="wp", bufs=1, name="wp_s")
            for h in range(NH):
                nc.sync.dma_start(yt_s[:, h], yt_d.ap()[h])
                nc.sync.dma_start(wp_s[:, h], wp_d.ap()[h])
            # h-outer accumulation: 8 resident psum tiles per ct-half so the
            # first matmuls only wait on wp[0]/yt[0]; lhsT shared across ct.
            for ch in range(2):
                pss = [[psp.tile([P, 512], f32, tag="ps", name="ps")
                        for _ in range(2)] for _ in range(4)]
                for h in range(NH):
                    for tt in range(4):
                        for c2 in range(2):
                            ct = ch * 2 + c2
                            nc.tensor.matmul(
                                pss[tt][c2], yt_s[:, h, tt * P:(tt + 1) * P],
                                wp_s[:, h, ct * 512:(ct + 1) * 512],
                                start=(h == 0), stop=(h == NH - 1))
                for tt in range(4):
                    for c2 in range(2):
                        ct = ch * 2 + c2
                        o_sb = sb.tile([P, 512], f32, tag="osb", bufs=4,
                                       name="o_sb")
                        nc.vector.tensor_copy(o_sb, pss[tt][c2])
                        nc.sync.dma_start(
                            out_d.ap()[tt * P:(tt + 1) * P,
                                       ct * 512:(ct + 1) * 512],
                            o_sb)
    nc.compile()
    return nc


_cache = {}


def kernel(x, freqs_cis, W_attn, W_proj, _trace=False, _timing=None):
    from concourse.bass_utils import run_bass_kernel_spmd

    per_core, _ = host_prep(x, freqs_cis, W_attn, W_proj)

    if "l1" not in _cache:
        _cache["l1"] = build_launch1()
    if "l2" not in _cache:
        _cache["l2"] = build_launch2()

    kw = dict(trace=True, trace_cores=list(range(NCORES))) if _trace else {}
    res1 = run_bass_kernel_spmd(_cache["l1"], per_core, list(range(NCORES)), **kw)
    yT_full = np.empty((NH, P, T), np.float16)
    for c in range(NCORES):
        yT_full[2 * c:2 * c + 2] = res1.results[c]["yt"]

    pc2 = host_prep_proj(yT_full, W_proj)
    res2 = run_bass_kernel_spmd(_cache["l2"], pc2, list(range(NCORES)), **kw)
    out = np.concatenate([res2.results[c]["out"] for c in range(NCORES)], axis=0)

    if _timing is not None:
        _timing["l1_ns"] = res1.exec_time_ns
        _timing["l2_ns"] = res2.exec_time_ns
        _timing["res1"] = res1
        _timing["res2"] = res2
    return out.reshape(B, T, C)


# revision 19
# speedup vs baseline: 1.4147x; 1.0667x over previous
"""Trainium2 Bass kernel for scrambled-GQA sliding-window attention.

Head-parallel across 8 NeuronCores, two SPMD launches, no collectives:
  launch 1: QKV projection + RoPE + banded attention -> per-core y^T (2 heads)
  launch 2: output projection, sequence-parallel rows -> per-core 512 output rows

All matmul operands are fp16 (1 col/cycle on PE, FWL weight loads, half DMA).
The two heads per core share K/V blocks, so score matmuls pair them as one
1024-column fp16 matmul (one PSUM bank).  Softmax denominators come from a
DVE-accumulated sum of exp tiles plus a single ones-matmul per (ti) tile
instead of one per block.  Q projection is interleaved with attention tiles
(tile ti only needs m-chunks 2ti, 2ti+1) to keep PE dense while ACT runs exp.

The torch-faithful "scrambled" reshapes in the reference are equivalent to
reinterpreting column slices of qkv = x @ W_attn:
  Q^T_h[d, m*256+t''] = qkv[t''*16+h, c_q(m)*128+d],  c_q(m)=m+2*(m//4), m in [0,16)
  K^T_h'[d, g*1024+u] = qkv[u*4+h', (6g+4)*128+d],    g in [0,4)
  V_h'[g*1024+u, d]   = qkv[u*4+h', (6g+5)*128+d]
Head h attends K/V block h' = h//4 over all rows with band |tq-tk| <= 1023.
RoPE applied to Q,K at position = row index (interleaved pairs).
"""

import math

import numpy as np

B, T, C = 1, 4096, 2048
NH, NKV, HD = 16, 4, 128
WINDOW = 1024
NCORES = 8
P = 128
KC = C // P            # 16 contraction chunks
NM = 16                # scramble chunks (m)
SCALE = 1.0 / math.sqrt(HD)

TQ = 512               # tq tile
NT = T // TQ           # 8 tiles per head
NBLK = T // P          # 32 tk blocks


def _cq(m):
    return m + 2 * (m // 4)


def _block_range(ti):
    b0 = 4 * ti
    return max(0, b0 - 8), min(NBLK - 1, b0 + 11)


def _mask_patterns():
    """Partial-band mask tiles keyed by D = tq0 - tkb (multiples of 128)."""
    ds = [640, 768, 896, 1024, -1024, -1152, -1280, -1408]
    tk = np.arange(P)[:, None]
    tq = np.arange(TQ)[None, :]
    masks = {}
    for d in ds:
        masks[d] = (np.abs(d + tq - tk) <= (WINDOW - 1)).astype(np.float32)
    return ds, masks


def host_prep(x, freqs_cis, W_attn, W_proj):
    """Build all per-core / shared numpy inputs for launch 1 (fp16)."""
    x = np.asarray(x, np.float32)
    freqs_cis = np.asarray(freqs_cis, np.float32)
    W_attn = np.asarray(W_attn, np.float32)

    xT = np.ascontiguousarray(x[0].T).astype(np.float16)    # (C, T)

    # RoPE tables, (128, T): rows 2i,2i+1 = cos(ang[:, i]); sin signed.
    cos = np.repeat(freqs_cis[:, :, 0].T, 2, axis=0).astype(np.float16)
    sin_base = freqs_cis[:, :, 1].T                          # (64, T)
    sin = np.empty((P, T), np.float32)
    sin[0::2] = -sin_base
    sin[1::2] = sin_base
    sin = sin.astype(np.float16)

    # pair-swap matrix (symmetric): row 2i <-> row 2i+1
    mt = np.zeros((P, P), np.float16)
    ii = np.arange(0, P, 2)
    mt[ii, ii + 1] = 1.0
    mt[ii + 1, ii] = 1.0

    mask_ds, masks = _mask_patterns()
    masks_arr = np.ascontiguousarray(
        np.stack([masks[d] for d in mask_ds], axis=1)).astype(np.float16)

    wa3 = W_attn.reshape(KC, P, 24, HD)          # [kc][p][blk][d]
    wq = np.stack(
        [np.ascontiguousarray(
            wa3[:, :, _cq(m), :].transpose(1, 0, 2).reshape(P, KC * HD))
         for m in range(NM)]
    ).astype(np.float16)                         # (16, 128, 2048)
    wk = np.stack(
        [np.ascontiguousarray(
            wa3[:, :, 6 * g + 4, :].transpose(1, 0, 2).reshape(P, KC * HD))
         for g in range(NKV)]
    ).astype(np.float16)                         # (4, 128, 2048)
    wv = np.ascontiguousarray(
        np.concatenate([wa3[:, :, 6 * g + 5, :] for g in range(NKV)], axis=2)
        .transpose(1, 0, 2)
        .reshape(P, KC, NKV * HD)
    ).astype(np.float16)                         # (128, 16, 512)

    per_core = []
    for c in range(NCORES):
        hp = c // 2
        cols = np.concatenate([np.arange(256) * 16 + (2 * c + z) for z in (0, 1)])
        xq = np.ascontiguousarray(
            xT[:, cols].reshape(KC, P, 512).transpose(1, 0, 2))   # (128, 16, 512)
        ucols = np.arange(1024) * 4 + hp
        xkv = np.ascontiguousarray(
            xT[:, ucols].reshape(KC, P, 1024).transpose(1, 0, 2))  # (128, 16, 1024)
        per_core.append(
            dict(xq=xq, xkv=xkv, wq=wq, wk=wk, wv=wv, cos=cos, sin=sin,
                 mt=mt, ones=np.ones((P, P), np.float16), masks=masks_arr)
        )
    return per_core, mask_ds


def host_prep_proj(yT_full, W_proj):
    """yT_full: (16, 128, 4096) fp16 per-head transposed attention output."""
    W_proj = np.asarray(W_proj, np.float32)
    wp = np.ascontiguousarray(W_proj.reshape(NH, HD, C)).astype(np.float16)
    per_core = []
    for c in range(NCORES):
        yt = np.ascontiguousarray(yT_full[:, :, c * 512:(c + 1) * 512])
        per_core.append(dict(yt=yt, wp=wp))
    return per_core


# ---------------------------------------------------------------------------
# numpy emulation of the exact device algorithm (validates all index math)
# ---------------------------------------------------------------------------

def emulate(x, freqs_cis, W_attn, W_proj):
    per_core, mask_ds = host_prep(x, freqs_cis, W_attn, W_proj)
    _, masks = _mask_patterns()
    yT_full = np.zeros((NH, P, T), np.float32)
    for c in range(NCORES):
        d = per_core[c]
        xq = d["xq"].astype(np.float32).transpose(1, 0, 2).reshape(C, 512)
        xkv = d["xkv"].astype(np.float32).transpose(1, 0, 2).reshape(C, 1024)
        cos = d["cos"].astype(np.float32)
        sin = d["sin"].astype(np.float32)
        mt = d["mt"].astype(np.float32)
        qr = np.zeros((2, P, T), np.float32)
        for m in range(NM):
            wq_full = d["wq"][m].astype(np.float32).reshape(P, KC, HD)\
                .transpose(1, 0, 2).reshape(C, HD)
            qt = wq_full.T @ xq                               # (128, 512)
            qsw = mt @ qt
            c2 = np.concatenate([cos[:, m * 256:(m + 1) * 256]] * 2, axis=1)
            s2 = np.concatenate([sin[:, m * 256:(m + 1) * 256]] * 2, axis=1)
            qt = qt * c2 + qsw * s2
            qr[0, :, m * 256:(m + 1) * 256] = qt[:, :256]
            qr[1, :, m * 256:(m + 1) * 256] = qt[:, 256:]
        kr = np.zeros((P, T), np.float32)
        for g in range(NKV):
            wkg = d["wk"][g].astype(np.float32).reshape(P, KC, HD)\
                .transpose(1, 0, 2).reshape(C, HD)
            kt = wkg.T @ xkv                                  # (128, 1024)
            ksw = mt @ kt
            sl = slice(g * 1024, (g + 1) * 1024)
            kr[:, sl] = kt * cos[:, sl] + ksw * sin[:, sl]
        vall = np.zeros((P, 8, 512), np.float32)
        wv_full = d["wv"].astype(np.float32).transpose(1, 0, 2).reshape(C, 512)
        for ut in range(8):
            vall[:, ut, :] = xkv[:, ut * 128:(ut + 1) * 128].T @ wv_full
        for z in range(2):
            for ti in range(NT):
                blo, bhi = _block_range(ti)
                q_tile = qr[z, :, ti * TQ:(ti + 1) * TQ]
                y_acc = np.zeros((P, TQ), np.float32)
                s_acc = np.zeros((TQ,), np.float32)
                for b in range(blo, bhi + 1):
                    st = kr[:, b * P:(b + 1) * P].T @ q_tile  # (128tk, 512)
                    pt = np.exp(SCALE * st)
                    D = 512 * ti - 128 * b
                    if D in masks:
                        pt = pt * masks[D]
                    g, ub = b // 8, b % 8
                    vblk = vall[:, ub, g * HD:(g + 1) * HD]   # (128u, 128d)
                    y_acc += vblk.T @ pt
                    s_acc += pt.sum(axis=0)
                yT_full[2 * c + z, :, ti * TQ:(ti + 1) * TQ] = y_acc / s_acc[None, :]
    pc = host_prep_proj(yT_full.astype(np.float16), W_proj)
    outs = []
    for c in range(NCORES):
        yt = pc[c]["yt"].astype(np.float32)
        wp = pc[c]["wp"].astype(np.float32)
        acc = np.zeros((512, C), np.float32)
        for h in range(NH):
            acc += yt[h].T @ wp[h]
        outs.append(acc)
    return np.concatenate(outs, axis=0).reshape(B, T, C)


# ---------------------------------------------------------------------------
# Bass programs
# ---------------------------------------------------------------------------

def build_launch1():
    import concourse.bacc as bacc
    import concourse.mybir as mybir
    import concourse.tile as tile

    import concourse.bass_isa as bass_isa

    f16 = mybir.dt.float16
    f32 = mybir.dt.float32
    MUL = mybir.AluOpType.mult
    ADD = mybir.AluOpType.add
    EXP = mybir.ActivationFunctionType.Exp

    nc = bacc.Bacc("TRN2", target_bir_lowering=False, debug=False)

    xq_d = nc.dram_tensor("xq", (P, KC, 512), f16, kind="ExternalInput")
    xkv_d = nc.dram_tensor("xkv", (P, KC, 1024), f16, kind="ExternalInput")
    wq_d = nc.dram_tensor("wq", (NM, P, KC * HD), f16, kind="ExternalInput")
    wk_d = nc.dram_tensor("wk", (NKV, P, KC * HD), f16, kind="ExternalInput")
    wv_d = nc.dram_tensor("wv", (P, KC, 512), f16, kind="ExternalInput")
    cos_d = nc.dram_tensor("cos", (P, T), f16, kind="ExternalInput")
    sin_d = nc.dram_tensor("sin", (P, T), f16, kind="ExternalInput")
    mt_d = nc.dram_tensor("mt", (P, P), f16, kind="ExternalInput")
    ones_d = nc.dram_tensor("ones", (P, P), f16, kind="ExternalInput")
    masks_d = nc.dram_tensor("masks", (P, 8, TQ), f16, kind="ExternalInput")
    yt_d = nc.dram_tensor("yt", (2, P, T), f16, kind="ExternalOutput")

    mask_ds, _ = _mask_patterns()
    mask_idx = {d: i for i, d in enumerate(mask_ds)}

    with tile.TileContext(nc) as tc:
        with tc.tile_pool(name="persist", bufs=1) as persist:
            qr = persist.tile([P, NT, 2, TQ], f16, tag="qr", name="qr")
            kr = persist.tile([P, T], f16, tag="kr", name="kr")
            vall = persist.tile([P, 8, TQ], f16, tag="vall", name="vall")
            ones = persist.tile([P, P], f16, tag="ones", name="ones")
            mt_s = persist.tile([P, P], f16, tag="mt", name="mt_s")
            masks_s = persist.tile([P, 8, TQ], f16, tag="masks", name="masks_s")
            xq_s = persist.tile([P, KC, 512], f16, tag="xq", name="xq_s")
            xkv_s = persist.tile([P, KC, 1024], f16, tag="xkv", name="xkv_s")

            with tc.tile_pool(name="wstream", bufs=2) as wsp:

                # ---- V (kc-outer: PE starts after first small DMA) ----
                with tc.tile_pool(name="vps", bufs=8, space="PSUM") as vps:
                    vpss = [vps.tile([P, TQ], f32, tag="vpsum",
                                     name="vpsum") for _ in range(8)]
                    for kc in range(KC):
                        nc.sync.dma_start(xkv_s[:, kc], xkv_d.ap()[:, kc])
                        wv_c = wsp.tile([P, 1, 512], f16, tag="wv",
                                        bufs=3, name="wv_c")
                        nc.sync.dma_start(wv_c[:, 0], wv_d.ap()[:, kc])
                        for ut in range(8):
                            nc.tensor.matmul(
                                vpss[ut],
                                xkv_s[:, kc, ut * P:(ut + 1) * P],
                                wv_c[:, 0],
                                start=(kc == 0), stop=(kc == KC - 1))
                    for ut in range(8):
                        nc.scalar.copy(vall[:, ut], vpss[ut])

                # prefetch wk while V finishes; xq / mt / masks after
                wk_ss = []
                for g in range(NKV):
                    wk_s = wsp.tile([P, KC, HD], f16, tag="wk", bufs=4,
                                    name="wk_s")
                    nc.sync.dma_start(
                        wk_s[:],
                        wk_d.ap()[g].rearrange("p (kc d) -> p kc d", d=HD))
                    wk_ss.append(wk_s)
                nc.sync.dma_start(mt_s[:], mt_d.ap())
                for kc in range(KC):
                    nc.sync.dma_start(xq_s[:, kc], xq_d.ap()[:, kc])
                nc.sync.dma_start(ones[:], ones_d.ap())
                nc.sync.dma_start(masks_s[:], masks_d.ap())

                def rope(ppool, sw_tag, sw_bufs, sb, tsl, n, out_ap, view, vtab):
                    """out = sb*cos + (M @ sb)*sin; sb is flat (128, 512) SBUF."""
                    ctab = wsp.tile([P, TQ], f16, tag="ctab", bufs=2, name="ctab")
                    nc.sync.dma_start(ctab[:, :n], cos_d.ap()[:, tsl])
                    stab = wsp.tile([P, TQ], f16, tag="stab", bufs=2, name="stab")
                    nc.sync.dma_start(stab[:, :n], sin_d.ap()[:, tsl])
                    sw = ppool.tile([P, TQ], f32, tag=sw_tag, bufs=sw_bufs,
                                    name=sw_tag)
                    nc.tensor.matmul(sw, mt_s[:], sb, start=True, stop=True)
                    t1 = wsp.tile([P, TQ], f16, tag="rope_t1", bufs=2,
                                  name="rope_t1")
                    nc.vector.tensor_tensor(view(t1), view(sb), vtab(ctab), MUL)
                    t2 = wsp.tile([P, TQ], f16, tag="rope_t2", bufs=2,
                                  name="rope_t2")
                    nc.vector.tensor_tensor(view(t2), view(sw), vtab(stab), MUL)
                    nc.vector.tensor_tensor(out_ap, view(t1), view(t2), ADD)

                # ---- K ----
                with tc.tile_pool(name="kqp", bufs=1, space="PSUM") as kqps:
                    for g in range(NKV):
                        for ut in range(2):
                            ps = kqps.tile([P, TQ], f32, tag="kps", bufs=2,
                                           name="kpsum")
                            for kc in range(KC):
                                nc.tensor.matmul(
                                    ps, wk_ss[g][:, kc],
                                    xkv_s[:, kc, ut * TQ:(ut + 1) * TQ],
                                    start=(kc == 0), stop=(kc == KC - 1))
                            ksb = wsp.tile([P, TQ], f16, tag="sbr", bufs=2,
                                           name="ksb")
                            nc.scalar.copy(ksb, ps)
                            sl = slice(g * 1024 + ut * TQ,
                                       g * 1024 + (ut + 1) * TQ)
                            rope(kqps, "ksw", 2, ksb, sl, TQ, kr[:, sl],
                                 lambda a: a, lambda tb: tb[:, :TQ])

                # ---- interleaved Q projection + attention ----
                # PSUM banks: big(2x2) + y(2) + qsw(1) + sq(1) = 8
                # Q for tile ti+1 is emitted mid-way through attention(ti) so
                # its rope (DVE) hides under attention matmuls.
                with tc.tile_pool(name="aps", bufs=1, space="PSUM") as aps, \
                     tc.tile_pool(name="asb", bufs=2) as asb:

                    def qproj(m):
                        wq_s = wsp.tile([P, KC, HD], f16, tag="wq", bufs=3,
                                        name="wq_s")
                        nc.sync.dma_start(
                            wq_s[:],
                            wq_d.ap()[m].rearrange("p (kc d) -> p kc d",
                                                   d=HD))
                        qps = aps.tile([P, 2, TQ], f32, tag="big", bufs=2,
                                       name="qpsum")
                        for kc in range(KC):
                            nc.tensor.matmul(
                                qps[:, 0], wq_s[:, kc], xq_s[:, kc],
                                start=(kc == 0), stop=(kc == KC - 1))
                        qsb = wsp.tile([P, TQ], f16, tag="sbr", bufs=2,
                                       name="qsb")
                        nc.scalar.copy(qsb, qps[:, 0])
                        ti, mh = m // 2, m % 2
                        rope(aps, "qsw", 1, qsb,
                             slice(m * 256, (m + 1) * 256), 256,
                             qr[:, ti, :, mh * 256:mh * 256 + 256],
                             lambda a: a.rearrange("p (z t) -> p z t", z=2),
                             lambda tb: tb[:, None, :256].broadcast_to(
                                 (P, 2, 256)))

                    qproj(0)
                    qproj(1)
                    for ti in range(NT):
                        blo, bhi = _block_range(ti)
                        bs = (list(range(4 * ti, bhi + 1))
                              + list(range(blo, 4 * ti)))
                        nb = len(bs)
                        q_rhs = qr[:, ti]                     # (128, 2, 512)
                        y_ps = aps.tile([P, 2, TQ], f32, tag="y", bufs=1,
                                        name="y_ps")
                        acc = asb.tile([P, 2, TQ], f16, tag="acc", bufs=2,
                                       name="acc")
                        nc.gpsimd.memset(acc[:], 0.0)

                        def pv(ent):
                            j, b, lo, hi, pt = ent
                            g, ub = b // 8, b % 8
                            for z in range(2):
                                nc.tensor.matmul(
                                    y_ps[:, z, lo:hi],
                                    vall[:, ub, g * HD:(g + 1) * HD],
                                    pt[:, z, lo:hi],
                                    start=(j == 0), stop=(j == nb - 1))

                        pend = []
                        for j, b in enumerate(bs):
                            if ti < NT - 1 and j in (4, 8):
                                qproj(2 * ti + 2 + (j == 8))
                            D = 512 * ti - 128 * b
                            # in-band tq range: |D + tq - tk| <= 1023
                            lo = max(0, -1023 - D)
                            hi = min(TQ, 1151 - D)
                            st = aps.tile([P, 2, TQ], f32, tag="big", bufs=2,
                                          name="st_ps")
                            for z in range(2):
                                nc.tensor.matmul(
                                    st[:, z, lo:hi], kr[:, b * P:(b + 1) * P],
                                    q_rhs[:, z, lo:hi], start=True, stop=True)
                            pt = asb.tile([P, 2, TQ], f16, tag="pt", bufs=4,
                                          name="pt")
                            nc.scalar.activation(pt[:, :, lo:hi],
                                                 st[:, :, lo:hi], EXP,
                                                 scale=SCALE)
                            if D in mask_idx:
                                nc.vector.tensor_tensor(
                                    pt[:, :, lo:hi], pt[:, :, lo:hi],
                                    masks_s[:, mask_idx[D]][:, None, lo:hi]
                                    .broadcast_to((P, 2, hi - lo)),
                                    MUL)
                            nc.vector.tensor_tensor(
                                acc[:, :, lo:hi], acc[:, :, lo:hi],
                                pt[:, :, lo:hi], ADD)
                            pend.append((j, b, lo, hi, pt))
                            if len(pend) > 2:
                                pv(pend.pop(0))
                        for ent in pend:
                            pv(ent)
                        r_sb = asb.tile([1, 2, TQ], f32, tag="rsb", bufs=2,
                                        name="r_sb")
                        rb_sb = asb.tile([P, 2, TQ], f32, tag="rb", bufs=2,
                                         name="rb_sb")
                        for z in range(2):
                            sq = aps.tile([1, TQ], f32, tag="sq", bufs=1,
                                          name="sq")
                            nc.tensor.matmul(sq, ones[:, 0:1], acc[:, z],
                                             start=True, stop=True)
                            nc.vector.reciprocal_approx_fast(r_sb[:, z], sq)
                            nc.gpsimd.partition_broadcast(rb_sb[:, z],
                                                          r_sb[:, z])
                        y_sb = asb.tile([P, 2, TQ], f16, tag="ysb", bufs=2,
                                        name="y_sb")
                        nc.vector.tensor_tensor(y_sb, y_ps, rb_sb, MUL)
                        for z in range(2):
                            nc.sync.dma_start(
                                yt_d.ap()[z, :, ti * TQ:(ti + 1) * TQ],
                                y_sb[:, z])

    nc.compile()
    return nc


def build_launch2():
    import concourse.bacc as bacc
    import concourse.mybir as mybir
    import concourse.tile as tile

    f16 = mybir.dt.float16
    f32 = mybir.dt.float32

    nc = bacc.Bacc("TRN2", target_bir_lowering=False, debug=False)
    yt_d = nc.dram_tensor("yt", (NH, P, 512), f16, kind="ExternalInput")
    wp_d = nc.dram_tensor("wp", (NH, P, C), f16, kind="ExternalInput")
    out_d = nc.dram_tensor("out", (512, C), f16, kind="ExternalOutput")

    with tile.TileContext(nc) as tc:
        with tc.tile_pool(name="sb", bufs=2) as sb, \
             tc.tile_pool(name="ps", bufs=8, space="PSUM") as psp:
            yt_s = sb.tile([P, NH, 512], f16, tag="yt", bufs=1, name="yt_s")
            wp_s = sb.tile([P, NH, C], f16, tag="wp", bufs=1, name="wp_s")
            for h in range(NH):
                nc.sync.dma_start(yt_s[:, h], yt_d.ap()[h])
                for q in range(2):
                    nc.sync.dma_start(wp_s[:, h, q * 1024:(q + 1) * 1024],
                                      wp_d.ap()[h, :, q * 1024:(q + 1) * 1024])
            # h-outer accumulation: 8 resident psum tiles per ct-half so the
            # first matmuls only wait on wp[0]/yt[0]; lhsT shared across ct.
            for ch in range(2):
                pss = [[psp.tile([P, 512], f32, tag="ps", name="ps")
                        for _ in range(2)] for _ in range(4)]
                for h in range(NH):
                    for tt in range(4):
                        for c2 in range(2):
                            ct = ch * 2 + c2
                            nc.tensor.matmul(
                                pss[tt][c2], yt_s[:, h, tt * P:(tt + 1) * P],
                                wp_s[:, h, ct * 512:(ct + 1) * 512],
                                start=(h == 0), stop=(h == NH - 1))
                for tt in range(4):
                    for c2 in range(2):
                        ct = ch * 2 + c2
                        o_sb = sb.tile([P, 512], f16, tag="osb", bufs=4,
                                       name="o_sb")
                        nc.vector.tensor_copy(o_sb, pss[tt][c2])
                        nc.sync.dma_start(
                            out_d.ap()[tt * P:(tt + 1) * P,
                                       ct * 512:(ct + 1) * 512],
                            o_sb)
    nc.compile()
    return nc


_cache = {}


def kernel(x, freqs_cis, W_attn, W_proj, _trace=False, _timing=None):
    from concourse.bass_utils import run_bass_kernel_spmd

    per_core, _ = host_prep(x, freqs_cis, W_attn, W_proj)

    if "l1" not in _cache:
        _cache["l1"] = build_launch1()
    if "l2" not in _cache:
        _cache["l2"] = build_launch2()

    kw = dict(trace=True, trace_cores=list(range(NCORES))) if _trace else {}
    res1 = run_bass_kernel_spmd(_cache["l1"], per_core, list(range(NCORES)), **kw)
    yT_full = np.empty((NH, P, T), np.float16)
    for c in range(NCORES):
        yT_full[2 * c:2 * c + 2] = res1.results[c]["yt"]

    pc2 = host_prep_proj(yT_full, W_proj)
    res2 = run_bass_kernel_spmd(_cache["l2"], pc2, list(range(NCORES)), **kw)
    out = np.concatenate([res2.results[c]["out"].astype(np.float32)
                          for c in range(NCORES)], axis=0)

    if _timing is not None:
        _timing["l1_ns"] = res1.exec_time_ns
        _timing["l2_ns"] = res2.exec_time_ns
        _timing["res1"] = res1
        _timing["res2"] = res2
    return out.reshape(B, T, C)
